# revision 11
# baseline (speedup 1.0000x reference)
"""Trainium2 Bass kernel for nn_MultiHeadAttention (B=2, S=2048, d_model=1024, H=16).

Sharding (8 cores): data-parallel over B (2) x tensor-parallel over head groups
(4 groups of 4 heads).  Each core computes its head-group's Q/K/V projections
(column-sharded weights), attention for its 4 heads, and a row-parallel
out_proj partial product.  The host sums the 4 partials per batch (the
"all-reduce") and adds the output bias.

Cost-model-driven design (matmul time = out-free-dim cycles, independent of
M/K; Ldweights engine-free):
  - scores computed transposed S_T[k, q] = Kh^T@Qh (kh stationary), exp'd in
    [128, 256] tiles split across THREE engines (ACT exact / DVE + Pool
    Schraudolph bit-trick) so no single engine's exp backlog stalls the
    PSUM-bank recycle that feeds the PE,
  - P@V uses the P tile as the STATIONARY operand and V as moving, with a
    ones-column appended to V: out[q, 65] per head -- column 64 is the
    softmax denominator for free,
  - normalization = ONE per-partition tensor_scalar divide (q on partitions),
    alternating DVE/Pool,
  - [q, e] -> [e, q] for the out_proj via the DMA XBAR transpose (14ns/tile
    on the DMA engines) instead of PE transpose matmuls + copies,
  - V projection computed directly non-transposed (vT tile stationary, Wv
    moving) -- no V transposes,
  - PE warmed up with throwaway matmuls during the initial DMA wait so the
    p-state ramp completes before real work arrives,
  - fp16 operands everywhere (fp32 PSUM accumulation).
"""

import sys
import numpy as np

for _p in ("/opt/trn_rl_repo", "/root/.axon_site/_ro/trn_rl_repo"):
    if _p not in sys.path:
        sys.path.append(_p)

D_MODEL = 1024
NUM_HEADS = 16
DK = 64
B = 2
S = 2048
N_CORES = 8
HPC = 4               # heads per core
E = HPC * DK          # 256 features per core
NQ = 256              # q-chunk size
N_QC = S // NQ        # 8 q chunks
N_KT = S // 128       # 16 k tiles
N_DT = D_MODEL // 128  # 8 contraction tiles for projections
EV = DK + 1           # V feature block incl. ones column (denominator)

_PROGRAM = None
_RUN_KWARGS = {}      # test harness may set {"trace": True}
_LAST_RESULTS = None  # BassKernelResults of the last run


def _build_program():
    import concourse.bass as bass
    import concourse.mybir as mybir
    from concourse import bacc, tile
    from contextlib import ExitStack

    f32 = mybir.dt.float32
    fp16 = mybir.dt.float16
    i16 = mybir.dt.int16
    AF = mybir.ActivationFunctionType
    ALU = mybir.AluOpType

    # exp split: ACT computes exact exp (with a bias that matches the
    # Schraudolph C1 factor); DVE and Pool compute a Schraudolph-style
    # bit-trick exp in ONE op (i16 = round(a*s + b), read back by the P@V
    # matmul through a bitcast-to-fp16 AP).  The bit pattern evaluates
    # C1*exp(s/8) with C1 = 1.04085 (mean mantissa-interpolation factor,
    # +-3% deviation); softmax cancels the common C1 row-wise.
    EXP_BIAS = float(np.log(1.0408461))
    SCHRA_A = 0.125 * 1024.0 / float(np.log(2.0))
    SCHRA_B = 15.0 * 1024.0

    # engine assignment table for exp tiles: 12 ACT / 12 DVE / 8 Pool per 32
    _t = list("ADP" * 11)[:32]
    _t[29] = "A"
    _t[26] = "D"
    EXP_TABLE = "".join(_t)

    nc = bacc.Bacc("TRN2", target_bir_lowering=False, debug=False,
                   num_devices=N_CORES)

    # Per-core DRAM I/O (transposed activations, pre-sliced weights)
    qT = nc.dram_tensor("qT", [D_MODEL, S], fp16, kind="ExternalInput").ap()
    kT = nc.dram_tensor("kT", [D_MODEL, S], fp16, kind="ExternalInput").ap()
    vT = nc.dram_tensor("vT", [D_MODEL, S], fp16, kind="ExternalInput").ap()
    wq = nc.dram_tensor("wq", [D_MODEL, E], fp16, kind="ExternalInput").ap()
    wk = nc.dram_tensor("wk", [D_MODEL, E], fp16, kind="ExternalInput").ap()
    wv = nc.dram_tensor("wv", [D_MODEL, E], fp16, kind="ExternalInput").ap()
    wo = nc.dram_tensor("wo", [E, D_MODEL], fp16, kind="ExternalInput").ap()
    bq = nc.dram_tensor("bq", [E, 1], f32, kind="ExternalInput").ap()
    bk = nc.dram_tensor("bk", [E, 1], f32, kind="ExternalInput").ap()
    bvb = nc.dram_tensor("bvb", [128, E], f32, kind="ExternalInput").ap()
    zT = nc.dram_tensor("zT", [D_MODEL, S], fp16, kind="ExternalOutput").ap()

    with tile.TileContext(nc) as tc, ExitStack() as ctx:
        persist = ctx.enter_context(tc.tile_pool(name="persist", bufs=1))
        const = ctx.enter_context(tc.tile_pool(name="const", bufs=1))

        # ---- weights + biases resident in SBUF --------------------------
        wq_sb = persist.tile([128, N_DT, E], fp16, tag="wq", name="wq")
        wk_sb = persist.tile([128, N_DT, E], fp16, tag="wk", name="wk")
        wv_sb = persist.tile([128, N_DT, E], fp16, tag="wv", name="wv")
        wo_sb = persist.tile([128, 2, D_MODEL], fp16, tag="wo", name="wo")
        bq_sb = persist.tile([128, 2], f32, tag="bq", name="bq")
        bk_sb = persist.tile([128, 2], f32, tag="bk", name="bk")
        bvb_sb = persist.tile([128, E], f32, tag="bvb", name="bvb")

        # PE warmup tile: all-zero fp16 operand for throwaway ramp matmuls
        wrm = const.tile([128, 128], fp16, tag="wrm", name="wrm")
        nc.gpsimd.memset(wrm[:], 0.0)

        # input stream tiles: [128, 2, 1024] half-pairs (two d-tiles per DMA)
        xpool = ctx.enter_context(tc.tile_pool(name="xpool", bufs=12))

        def load_half_pair(src, d, h):
            # two d-tiles in one DMA: halves the HWDGE issue cost on the
            # startup-critical k/q half-0 stream
            xt = xpool.tile([128, 2, S // 2], fp16, tag="xt2", name="xt2")
            nc.sync.dma_start(
                xt[:], src.rearrange("(t p) s -> p t s", p=128)[
                    :, d:d + 2, h * (S // 2):(h + 1) * (S // 2)])
            return [xt[:, 0, :], xt[:, 1, :]]

        # startup-critical loads first: wk d0 slice, then kT half0 in 2d
        # pairs chased by the lead-in projection, then wq + qT half0 ...
        wkv = wk.rearrange("(t p) e -> p t e", p=128)
        kts = [[None, None] for _ in range(N_DT)]
        qts = [[None, None] for _ in range(N_DT)]
        vts = [[None, None] for _ in range(N_DT)]

        nc.sync.dma_start(wk_sb[:, 0:1, :], wkv[:, 0:1, :])
        kts[0][0], kts[1][0] = load_half_pair(kT, 0, 0)
        nc.sync.dma_start(wk_sb[:, 1:4, :], wkv[:, 1:4, :])
        kts[2][0], kts[3][0] = load_half_pair(kT, 2, 0)
        nc.sync.dma_start(wk_sb[:, 4:8, :], wkv[:, 4:8, :])
        kts[4][0], kts[5][0] = load_half_pair(kT, 4, 0)
        kts[6][0], kts[7][0] = load_half_pair(kT, 6, 0)
        nc.sync.dma_start(wq_sb[:], wq.rearrange("(t p) e -> p t e", p=128))
        qts[0][0], qts[1][0] = load_half_pair(qT, 0, 0)
        qts[2][0], qts[3][0] = load_half_pair(qT, 2, 0)
        qts[4][0], qts[5][0] = load_half_pair(qT, 4, 0)
        qts[6][0], qts[7][0] = load_half_pair(qT, 6, 0)
        nc.sync.dma_start(bk_sb[:], bk.rearrange("(m p) o -> p (m o)", p=128))
        nc.sync.dma_start(bq_sb[:], bq.rearrange("(m p) o -> p (m o)", p=128))
        for d in range(0, N_DT, 2):
            kts[d][1], kts[d + 1][1] = load_half_pair(kT, d, 1)
        nc.sync.dma_start(wv_sb[:], wv.rearrange("(t p) e -> p t e", p=128))
        nc.sync.dma_start(bvb_sb[:], bvb)
        for d in range(0, N_DT, 2):
            vts[d][0], vts[d + 1][0] = load_half_pair(vT, d, 0)
        for d in range(0, N_DT, 2):
            vts[d][1], vts[d + 1][1] = load_half_pair(vT, d, 1)
        for d in range(0, N_DT, 2):
            qts[d][1], qts[d + 1][1] = load_half_pair(qT, d, 1)
        nc.sync.dma_start(wo_sb[:], wo.rearrange("(t p) e -> p t e", p=128))

        ebias = const.tile([128, 1], f32, tag="ebias", name="ebias")
        nc.gpsimd.memset(ebias[:], EXP_BIAS)

        # ---- persistent activations ------------------------------------
        qh = [persist.tile([128, S], fp16, tag=f"qh{p}", name=f"qh{p}")
              for p in range(2)]
        kh = [persist.tile([128, S], fp16, tag=f"kh{p}", name=f"kh{p}")
              for p in range(2)]
        # V projection non-transposed: [s(128-tile), kt, head, 65]
        vh65 = persist.tile([128, N_KT * HPC * EV], fp16, tag="vh65",
                            name="vh65")
        vh65v = vh65.rearrange("p (t h c) -> p t h c", t=N_KT, h=HPC)
        # ones columns for the softmax denominators
        nc.gpsimd.memset(vh65v[:, :, :, DK:DK + 1], 1.0)

        # normalized attention out per chunk-pair, laid out for the XBAR
        # transpose: [q(128), qslot(4), e(256)]
        ocp = ctx.enter_context(tc.tile_pool(name="ocp", bufs=2))
        # transposed: [e(128), cc(2), qslot(4), q(128)]
        otTp = ctx.enter_context(tc.tile_pool(name="otTp", bufs=2))

        # ---- PSUM pools (exactly 8 banks) ------------------------------
        # HW rule: one (non-transpose) matmul output region per PSUM bank
        # (partition-splits may share; column-splits may not).
        scorep = ctx.enter_context(
            tc.tile_pool(name="scorep", bufs=4, space="PSUM"))  # 4x1 bank
        pvp = ctx.enter_context(
            tc.tile_pool(name="pvp", bufs=2, space="PSUM"))     # 2x1 bank
        miscp = ctx.enter_context(
            tc.tile_pool(name="miscp", bufs=2, space="PSUM"))   # 2x1 bank

        ptp = ctx.enter_context(tc.tile_pool(name="ptp", bufs=40))
        zsbp = ctx.enter_context(tc.tile_pool(name="zsbp", bufs=2))

        # ---- PE p-state warmup ------------------------------------------
        # ~30 throwaway [128,128] matmuls run during the initial DMA wait so
        # the PE clock ramps to full speed before real work arrives.
        wps = miscp.tile([128, 128], f32, tag="misc", name="wps")
        for _ in range(30):
            nc.tensor.matmul(wps[:], wrm[:], wrm[:], start=True, stop=True,
                             skip_group_check=True)

        # ---- projection helpers ----------------------------------------
        def proj_pair(xhalves, w_sb, b_sb, dst, nb):
            # both m accumulation groups of a seq block, d-outer so the
            # d-step stream chases the input DMA arrivals; psum evacuated
            # (with bias) on ACT
            xh, off = nb // 2, (nb % 2) * 512
            ps = [miscp.tile([128, 512], f32, tag="misc", name="pps")
                  for _ in range(2)]
            for d in range(N_DT):
                for m in range(2):
                    nc.tensor.matmul(
                        ps[m][:], w_sb[:, d, m * 128:(m + 1) * 128],
                        xhalves[d][xh][:, off:off + 512],
                        start=(d == 0), stop=(d == N_DT - 1))
            for m in range(2):
                nc.scalar.activation(
                    dst[m][:, nb * 512:(nb + 1) * 512], ps[m][:],
                    AF.Identity, bias=b_sb[:, m:m + 1])

        def proj_v_st(st):
            # V projection, direct: out [s(128), e(256)] for s-tile st;
            # psum evacuated (with bias) alternating DVE/Pool
            xh = st // 8
            ps = miscp.tile([128, 512], f32, tag="misc", name="vps")
            for d in range(N_DT):
                nc.tensor.matmul(
                    ps[:, 0:E],
                    vts[d][xh][:, (st % 8) * 128:(st % 8) * 128 + 128],
                    wv_sb[:, d, :], start=(d == 0), stop=(d == N_DT - 1))
            eng = nc.vector if st % 2 == 0 else nc.gpsimd
            eng.tensor_tensor(
                vh65v[:, st, :, 0:DK],
                ps[:, 0:E].rearrange("p (h j) -> p h j", h=HPC),
                bvb_sb.rearrange("p (h j) -> p h j", h=HPC),
                op=mybir.AluOpType.add)

        # ---- per-slot emission pieces ----------------------------------
        zsb = {}
        pts_of = {}   # chunk -> list of pt tiles (one per kt)
        pv_of = {}    # (chunk, sweep) -> pv psum tile [128, 65]
        oc_of = {}    # chunk-pair -> oc tile [128, 4, 256]
        otT_of = {}   # chunk-pair -> otT tile [128, 2, 4, 128]
        exp_idx = [0]

        def emit_scores_exp_kt(c, kt):
            # two heads COOPERATE in one [128, 512] PSUM bank: the first
            # matmul (start=True) zeroes the whole 2KB zero-region, the
            # second accumulates into the untouched half (PE is in-order so
            # the pair cannot race); ONE exp call then drains the full bank.
            # Halves both the per-bank recycle chain (8 tiles/bank/step) and
            # the exp call count.  exp assigned by table across ACT/DVE/Pool.
            pt = ptp.tile([128, HPC * NQ], fp16, tag="pt", name="pt")
            for hp in range(2):
                sc = scorep.tile([128, 2 * NQ], f32, tag="sc", name="sc")
                for j in range(2):
                    nc.tensor.matmul(
                        sc[:, j * NQ:(j + 1) * NQ],
                        kh[hp][j * 64:(j + 1) * 64, kt * 128:(kt + 1) * 128],
                        qh[hp][j * 64:(j + 1) * 64, c * NQ:(c + 1) * NQ],
                        start=(j == 0), stop=(j == 1), skip_group_check=True)
                dst = pt[:, hp * 2 * NQ:(hp + 1) * 2 * NQ]
                eng = EXP_TABLE[exp_idx[0] % 32]
                exp_idx[0] += 1
                if eng == "A":
                    nc.scalar.activation(dst, sc[:], AF.Exp, scale=0.125,
                                         bias=ebias[:])
                else:
                    veng = nc.vector if eng == "D" else nc.gpsimd
                    veng.tensor_scalar(
                        dst.bitcast(i16), sc[:], SCHRA_A, SCHRA_B,
                        op0=ALU.mult, op1=ALU.add)
            pts_of.setdefault(c, []).append(pt)

        def emit_pv_half(c, s, half):
            # P@V sweep for (qb = s//4, head = s%4): 8 accumulating
            # matmuls (k-tiles half*8..half*8+7) into one [128, 65] bank
            qb, h = s // 4, s % 4
            if half == 0:
                pv_of[(c, s)] = pvp.tile([128, EV], f32, tag="pv", name="pv")
            pv = pv_of[(c, s)]
            pts = pts_of[c]
            for kt in range(half * 8, half * 8 + 8):
                nc.tensor.matmul(
                    pv[:],
                    pts[kt][:, h * NQ + qb * 128:h * NQ + qb * 128 + 128],
                    vh65v[:, kt, h, :],
                    start=(kt == 0), stop=(kt == N_KT - 1),
                    skip_group_check=True)

        def emit_norm_s(c, s):
            # ONE tensor_scalar divide by the denominator column, written
            # straight into the XBAR-transpose-ready oc layout
            qb, h = s // 4, s % 4
            pv = pv_of.pop((c, s))
            cp, qslot = c // 2, (c % 2) * 2 + qb
            if cp not in oc_of:
                oc_of[cp] = ocp.tile([128, 4, E], fp16, tag="oc", name="oc")
            dst = oc_of[cp][:, qslot, h * DK:(h + 1) * DK]
            eng = nc.vector if (c + s) % 2 == 0 else nc.gpsimd
            eng.tensor_scalar(dst, pv[:, 0:DK], pv[:, DK:DK + 1], None,
                              op0=ALU.divide)

        def emit_otT_dma(c, qb):
            # [q, e] -> [e, q] via the DMA XBAR: one call transposes both
            # 128-wide e-halves of (c, qb) into otT[:, :, qslot, :]
            cp, qslot = c // 2, (c % 2) * 2 + qb
            if cp not in otT_of:
                otT_of[cp] = otTp.tile([128, 2, 4, 128], fp16, tag="otT",
                                       name="otT")
            nc.sync.dma_start(otT_of[cp][:, :, qslot, :],
                              oc_of[cp][:, qslot, :], transpose=True)
            if qb == 1:
                pts_of.pop(c, None)
                if c % 2 == 1:
                    oc_of.pop(cp, None)

        def emit_outproj(cp, eo, qh):
            # out_proj partial for chunk-pair cp, qslot-half qh (256 q):
            # zT[eo-block, q-half].  qh0 (the even chunk's qslots) runs a
            # full 12 slots earlier than qh1 so the PE stays fed while the
            # odd chunk's P@V/norm/otT chain drains.  Evac spread over
            # ACT/DVE/Pool; zT DMA'd per 2-eo piece to shorten the tail.
            otT = otT_of[cp]
            zps = miscp.tile([128, 256], f32, tag="misc", name="zps")
            for cc in range(2):
                nc.tensor.matmul(
                    zps[:], wo_sb[:, cc, eo * 128:(eo + 1) * 128],
                    otT[:, cc, 2 * qh:2 * qh + 2, :],
                    start=(cc == 0), stop=(cc == 1), skip_group_check=True)
            if eo == 0 and qh == 0:
                zsb[cp] = zsbp.tile([128, 8, 2 * NQ], fp16, tag="zsb",
                                    name="zs")
            dst = zsb[cp][:, eo, qh * 256:(qh + 1) * 256]
            sel = (eo * 2 + qh) % 4
            if sel in (0, 2):
                nc.scalar.activation(dst, zps[:], AF.Copy)
            elif sel == 1:
                nc.vector.tensor_copy(dst, zps[:])
            else:
                nc.gpsimd.tensor_copy(dst, zps[:])
            if qh == 1 and eo % 2 == 1:
                nc.sync.dma_start(
                    zT.rearrange("(eo p) s -> p eo s", p=128)[
                        :, eo - 1:eo + 1, cp * 2 * NQ:(cp + 1) * 2 * NQ],
                    zsb[cp][:, eo - 1:eo + 1, :])
            if eo == 7 and qh == 1:
                otT_of.pop(cp, None)

        # ---- software pipeline -----------------------------------------
        # lead-in: k half0 projection + first q block
        proj_pair(kts, wk_sb, bk_sb, kh, 0)
        proj_pair(qts, wq_sb, bq_sb, qh, 0)
        proj_pair(kts, wk_sb, bk_sb, kh, 1)

        extras = {}

        def add_extra(cs, kt, fn):
            extras.setdefault((cs, kt), []).append(fn)

        # chunk0: k half1 at kt4/6 (needed by scores kt8+),
        #         V st0-7 at kt8-15, V st8-15 at step1 kt0-7 (needed by
        #         the PV(c0) sweeps which start at step1 slot 8)
        add_extra(0, 4, lambda: proj_pair(kts, wk_sb, bk_sb, kh, 2))
        add_extra(0, 6, lambda: proj_pair(kts, wk_sb, bk_sb, kh, 3))
        for i in range(8):
            add_extra(0, 8 + i, lambda st=i: proj_v_st(st))
        for i in range(8):
            add_extra(1, i, lambda st=8 + i: proj_v_st(st))
        add_extra(1, 8, lambda: proj_pair(qts, wq_sb, bq_sb, qh, 1))
        add_extra(2, 5, lambda: proj_pair(qts, wq_sb, bq_sb, qh, 2))
        add_extra(3, 5, lambda: proj_pair(qts, wq_sb, bq_sb, qh, 3))

        # Schedule per step/slot.  Sweeps of chunk c: qb0 (s 0-3) at step
        # c+1 slots 8-15, qb1 (s 4-7) at step c+2 slots 0-7; norms trail
        # each sweep; otT XBAR-transpose DMAs at step c+2 slots 1/9;
        # out_proj per chunk-pair at the odd chunk's step+2, slots 12-15.
        for step in range(N_QC + 2):
            for kt in range(N_KT):
                # qb1 sweeps + trailing norms for chunk step-2
                c2 = step - 2
                if 0 <= c2 <= N_QC - 1:
                    if kt < 8:
                        emit_pv_half(c2, 4 + kt // 2, kt % 2)
                    if kt == 0:
                        emit_norm_s(c2, 3)
                    if kt in (2, 4, 6, 8) and kt // 2 + 3 <= 7:
                        emit_norm_s(c2, kt // 2 + 3)
                    if kt == 1:
                        emit_otT_dma(c2, 0)
                    if kt == 9:
                        emit_otT_dma(c2, 1)
                    if c2 % 2 == 1 and kt <= 3:
                        emit_outproj(c2 // 2, 2 * kt, 0)
                        emit_outproj(c2 // 2, 2 * kt + 1, 0)
                    if c2 % 2 == 1 and 12 <= kt <= 15:
                        emit_outproj(c2 // 2, 2 * (kt - 12), 1)
                        emit_outproj(c2 // 2, 2 * (kt - 12) + 1, 1)
                # qb0 sweeps + norms for chunk step-1
                c1 = step - 1
                if 0 <= c1 <= N_QC - 1 and kt >= 8:
                    emit_pv_half(c1, (kt - 8) // 2, kt % 2)
                    if kt in (10, 12, 14):
                        emit_norm_s(c1, (kt - 10) // 2)
                for fn in extras.get((step, kt), ()):
                    fn()
                if step <= N_QC - 1:
                    emit_scores_exp_kt(step, kt)

    nc.compile()
    return nc


def _get_program():
    global _PROGRAM
    if _PROGRAM is None:
        _PROGRAM = _build_program()
    return _PROGRAM


def _make_in_maps(q, k, v, Wq, bq, Wk, bk, Wv, bv, Wo):
    f32 = np.float32
    xT = {}
    for b in range(B):
        xT[("q", b)] = np.ascontiguousarray(q[b].T, dtype=np.float16)
        xT[("k", b)] = np.ascontiguousarray(k[b].T, dtype=np.float16)
        xT[("v", b)] = np.ascontiguousarray(v[b].T, dtype=np.float16)
    wslices = {}
    for g in range(4):
        sl = slice(g * E, (g + 1) * E)
        wslices[("wq", g)] = np.ascontiguousarray(Wq[sl, :].T, dtype=np.float16)
        wslices[("wk", g)] = np.ascontiguousarray(Wk[sl, :].T, dtype=np.float16)
        wslices[("wv", g)] = np.ascontiguousarray(Wv[sl, :].T, dtype=np.float16)
        wslices[("wo", g)] = np.ascontiguousarray(Wo[:, sl].T, dtype=np.float16)
        wslices[("bq", g)] = np.ascontiguousarray(bq[sl].reshape(E, 1), dtype=f32)
        wslices[("bk", g)] = np.ascontiguousarray(bk[sl].reshape(E, 1), dtype=f32)
        wslices[("bvb", g)] = np.ascontiguousarray(
            np.tile(bv[sl].reshape(1, E), (128, 1)), dtype=f32)
    in_maps = []
    for c in range(N_CORES):
        b, g = c // 4, c % 4
        in_maps.append({
            "qT": xT[("q", b)], "kT": xT[("k", b)], "vT": xT[("v", b)],
            "wq": wslices[("wq", g)], "wk": wslices[("wk", g)],
            "wv": wslices[("wv", g)], "wo": wslices[("wo", g)],
            "bq": wslices[("bq", g)], "bk": wslices[("bk", g)],
            "bvb": wslices[("bvb", g)],
        })
    return in_maps


def _numpy_fallback(q, k, v, mask, Wq, bq, Wk, bk, Wv, bv, Wo, bo):
    # Only used if mask is not all-True (never the case for this problem).
    def proj(x, W, b_):
        y = x @ W.T + b_
        return y.reshape(B, S, NUM_HEADS, DK).transpose(0, 2, 1, 3)
    qh, kh, vh = proj(q, Wq, bq), proj(k, Wk, bk), proj(v, Wv, bv)
    sc = np.einsum("bhqd,bhkd->bhqk", qh, kh) / np.sqrt(DK)
    sc = np.where(mask, sc, np.float32(-1e9))
    sc = sc - sc.max(-1, keepdims=True)
    p = np.exp(sc)
    p /= p.sum(-1, keepdims=True)
    o = np.einsum("bhqk,bhkd->bhqd", p, vh)
    o = o.transpose(0, 2, 1, 3).reshape(B, S, D_MODEL)
    return (o @ Wo.T + bo).astype(np.float32)


def kernel(q, k, v, mask, Wq, bq, Wk, bk, Wv, bv, Wo, bo):
    q = np.asarray(q, dtype=np.float32)
    k = np.asarray(k, dtype=np.float32)
    v = np.asarray(v, dtype=np.float32)
    Wq, Wk, Wv, Wo = (np.asarray(w, dtype=np.float32) for w in (Wq, Wk, Wv, Wo))
    bq, bk, bv, bo = (np.asarray(x, dtype=np.float32) for x in (bq, bk, bv, bo))
    if not np.all(np.asarray(mask)):
        return _numpy_fallback(q, k, v, np.asarray(mask), Wq, bq, Wk, bk,
                               Wv, bv, Wo, bo)

    from concourse.bass_utils import run_bass_kernel_spmd
    nc = _get_program()
    in_maps = _make_in_maps(q, k, v, Wq, bq, Wk, bk, Wv, bv, Wo)
    res = run_bass_kernel_spmd(nc, in_maps, core_ids=list(range(N_CORES)),
                               **_RUN_KWARGS)
    global _LAST_RESULTS
    _LAST_RESULTS = res
    out = np.empty((B, S, D_MODEL), dtype=np.float32)
    for b in range(B):
        acc = res.results[4 * b]["zT"].astype(np.float32)
        for g in range(1, 4):
            acc = acc + res.results[4 * b + g]["zT"].astype(np.float32)
        out[b] = acc.T + bo
    return out


# revision 12
# speedup vs baseline: 1.0490x; 1.0490x over previous
"""Trainium2 Bass kernel for nn_MultiHeadAttention (B=2, S=2048, d_model=1024, H=16).

Sharding (8 cores): data-parallel over B (2) x tensor-parallel over head groups
(4 groups of 4 heads).  Each core computes its head-group's Q/K/V projections
(column-sharded weights), attention for its 4 heads, and a row-parallel
out_proj partial product.  The host sums the 4 partials per batch (the
"all-reduce") and adds the output bias.

Cost-model-driven design (matmul time = out-free-dim cycles, independent of
M/K; Ldweights engine-free):
  - scores computed transposed S_T[k, q] = Kh^T@Qh (kh stationary), exp'd in
    [128, 256] tiles split across THREE engines (ACT exact / DVE + Pool
    Schraudolph bit-trick) so no single engine's exp backlog stalls the
    PSUM-bank recycle that feeds the PE,
  - P@V uses the P tile as the STATIONARY operand and V as moving, with a
    ones-column appended to V: out[q, 65] per head -- column 64 is the
    softmax denominator for free,
  - normalization = ONE per-partition tensor_scalar divide (q on partitions),
    alternating DVE/Pool,
  - [q, e] -> [e, q] for the out_proj via the DMA XBAR transpose (14ns/tile
    on the DMA engines) instead of PE transpose matmuls + copies,
  - V projection computed directly non-transposed (vT tile stationary, Wv
    moving) -- no V transposes,
  - PE warmed up with throwaway matmuls during the initial DMA wait so the
    p-state ramp completes before real work arrives,
  - fp16 operands everywhere (fp32 PSUM accumulation).
"""

import sys
import numpy as np

for _p in ("/opt/trn_rl_repo", "/root/.axon_site/_ro/trn_rl_repo"):
    if _p not in sys.path:
        sys.path.append(_p)

D_MODEL = 1024
NUM_HEADS = 16
DK = 64
B = 2
S = 2048
N_CORES = 8
HPC = 4               # heads per core
E = HPC * DK          # 256 features per core
NQ = 256              # q-chunk size
N_QC = S // NQ        # 8 q chunks
N_KT = S // 128       # 16 k tiles
N_DT = D_MODEL // 128  # 8 contraction tiles for projections
EV = DK + 1           # V feature block incl. ones column (denominator)

_PROGRAM = None
_RUN_KWARGS = {}      # test harness may set {"trace": True}
_LAST_RESULTS = None  # BassKernelResults of the last run


def _build_program():
    import concourse.bass as bass
    import concourse.mybir as mybir
    from concourse import bacc, tile
    from contextlib import ExitStack

    f32 = mybir.dt.float32
    fp16 = mybir.dt.float16
    i16 = mybir.dt.int16
    AF = mybir.ActivationFunctionType
    ALU = mybir.AluOpType

    # exp split: ACT computes exact exp (with a bias that matches the
    # Schraudolph C1 factor); DVE and Pool compute a Schraudolph-style
    # bit-trick exp in ONE op (i16 = round(a*s + b), read back by the P@V
    # matmul through a bitcast-to-fp16 AP).  The bit pattern evaluates
    # C1*exp(s/8) with C1 = 1.04085 (mean mantissa-interpolation factor,
    # +-3% deviation); softmax cancels the common C1 row-wise.
    EXP_BIAS = float(np.log(1.0408461))
    SCHRA_A = 0.125 * 1024.0 / float(np.log(2.0))
    SCHRA_B = 15.0 * 1024.0

    # engine assignment table for exp tiles: 12 ACT / 12 DVE / 8 Pool per 32
    _t = list("ADP" * 11)[:32]
    _t[29] = "A"
    _t[26] = "D"
    EXP_TABLE = "".join(_t)

    nc = bacc.Bacc("TRN2", target_bir_lowering=False, debug=False,
                   num_devices=N_CORES)

    # Per-core DRAM I/O (transposed activations, pre-sliced weights)
    qT = nc.dram_tensor("qT", [D_MODEL, S], fp16, kind="ExternalInput").ap()
    kT = nc.dram_tensor("kT", [D_MODEL, S], fp16, kind="ExternalInput").ap()
    vT = nc.dram_tensor("vT", [D_MODEL, S], fp16, kind="ExternalInput").ap()
    wq = nc.dram_tensor("wq", [D_MODEL, E], fp16, kind="ExternalInput").ap()
    wk = nc.dram_tensor("wk", [D_MODEL, E], fp16, kind="ExternalInput").ap()
    wv = nc.dram_tensor("wv", [D_MODEL, E], fp16, kind="ExternalInput").ap()
    wo = nc.dram_tensor("wo", [E, D_MODEL], fp16, kind="ExternalInput").ap()
    bq = nc.dram_tensor("bq", [E, 1], f32, kind="ExternalInput").ap()
    bk = nc.dram_tensor("bk", [E, 1], f32, kind="ExternalInput").ap()
    bvb = nc.dram_tensor("bvb", [128, E], f32, kind="ExternalInput").ap()
    zT = nc.dram_tensor("zT", [D_MODEL, S], fp16, kind="ExternalOutput").ap()

    with tile.TileContext(nc) as tc, ExitStack() as ctx:
        persist = ctx.enter_context(tc.tile_pool(name="persist", bufs=1))
        const = ctx.enter_context(tc.tile_pool(name="const", bufs=1))

        # ---- weights + biases resident in SBUF --------------------------
        wq_sb = persist.tile([128, N_DT, E], fp16, tag="wq", name="wq")
        wk_sb = persist.tile([128, N_DT, E], fp16, tag="wk", name="wk")
        wv_sb = persist.tile([128, N_DT, E], fp16, tag="wv", name="wv")
        wo_sb = persist.tile([128, 2, D_MODEL], fp16, tag="wo", name="wo")
        bq_sb = persist.tile([128, 2], f32, tag="bq", name="bq")
        bk_sb = persist.tile([128, 2], f32, tag="bk", name="bk")
        bvb_sb = persist.tile([128, E], f32, tag="bvb", name="bvb")

        # PE warmup tile: all-zero fp16 operand for throwaway ramp matmuls
        wrm = const.tile([128, 128], fp16, tag="wrm", name="wrm")
        nc.gpsimd.memset(wrm[:], 0.0)

        # input stream tiles: [128, 2, 1024] half-pairs (two d-tiles per DMA)
        xpool = ctx.enter_context(tc.tile_pool(name="xpool", bufs=12))

        def load_half_pair(src, d, h):
            # two d-tiles in one DMA: halves the HWDGE issue cost on the
            # startup-critical k/q half-0 stream
            xt = xpool.tile([128, 2, S // 2], fp16, tag="xt2", name="xt2")
            nc.sync.dma_start(
                xt[:], src.rearrange("(t p) s -> p t s", p=128)[
                    :, d:d + 2, h * (S // 2):(h + 1) * (S // 2)])
            return [xt[:, 0, :], xt[:, 1, :]]

        # startup-critical loads first: wk d0 slice, then kT half0 in 2d
        # pairs chased by the lead-in projection, then wq + qT half0 ...
        wkv = wk.rearrange("(t p) e -> p t e", p=128)
        kts = [[None, None] for _ in range(N_DT)]
        qts = [[None, None] for _ in range(N_DT)]
        vts = [[None, None] for _ in range(N_DT)]

        nc.sync.dma_start(wk_sb[:, 0:1, :], wkv[:, 0:1, :])
        kts[0][0], kts[1][0] = load_half_pair(kT, 0, 0)
        nc.sync.dma_start(wk_sb[:, 1:4, :], wkv[:, 1:4, :])
        kts[2][0], kts[3][0] = load_half_pair(kT, 2, 0)
        nc.sync.dma_start(wk_sb[:, 4:8, :], wkv[:, 4:8, :])
        nc.sync.dma_start(bk_sb[:], bk.rearrange("(m p) o -> p (m o)", p=128))
        kts[4][0], kts[5][0] = load_half_pair(kT, 4, 0)
        kts[6][0], kts[7][0] = load_half_pair(kT, 6, 0)
        nc.sync.dma_start(wq_sb[:], wq.rearrange("(t p) e -> p t e", p=128))
        nc.sync.dma_start(bq_sb[:], bq.rearrange("(m p) o -> p (m o)", p=128))
        qts[0][0], qts[1][0] = load_half_pair(qT, 0, 0)
        qts[2][0], qts[3][0] = load_half_pair(qT, 2, 0)
        qts[4][0], qts[5][0] = load_half_pair(qT, 4, 0)
        qts[6][0], qts[7][0] = load_half_pair(qT, 6, 0)
        for d in range(0, N_DT, 2):
            kts[d][1], kts[d + 1][1] = load_half_pair(kT, d, 1)
        nc.sync.dma_start(wv_sb[:], wv.rearrange("(t p) e -> p t e", p=128))
        nc.sync.dma_start(bvb_sb[:], bvb)
        for d in range(0, N_DT, 2):
            vts[d][0], vts[d + 1][0] = load_half_pair(vT, d, 0)
        for d in range(0, N_DT, 2):
            vts[d][1], vts[d + 1][1] = load_half_pair(vT, d, 1)
        for d in range(0, N_DT, 2):
            qts[d][1], qts[d + 1][1] = load_half_pair(qT, d, 1)
        nc.sync.dma_start(wo_sb[:], wo.rearrange("(t p) e -> p t e", p=128))

        ebias = const.tile([128, 1], f32, tag="ebias", name="ebias")
        nc.gpsimd.memset(ebias[:], EXP_BIAS)

        # ---- persistent activations ------------------------------------
        qh = [persist.tile([128, S], fp16, tag=f"qh{p}", name=f"qh{p}")
              for p in range(2)]
        kh = [persist.tile([128, S], fp16, tag=f"kh{p}", name=f"kh{p}")
              for p in range(2)]
        # V projection non-transposed: [s(128-tile), kt, head, 65]
        vh65 = persist.tile([128, N_KT * HPC * EV], fp16, tag="vh65",
                            name="vh65")
        vh65v = vh65.rearrange("p (t h c) -> p t h c", t=N_KT, h=HPC)
        # ones columns for the softmax denominators
        nc.gpsimd.memset(vh65v[:, :, :, DK:DK + 1], 1.0)

        # normalized attention out per chunk-pair, laid out for the XBAR
        # transpose: [q(128), qslot(4), e(256)]
        ocp = ctx.enter_context(tc.tile_pool(name="ocp", bufs=2))
        # transposed: [e(128), cc(2), qslot(4), q(128)]
        otTp = ctx.enter_context(tc.tile_pool(name="otTp", bufs=2))

        # ---- PSUM pools (exactly 8 banks) ------------------------------
        # HW rule: one (non-transpose) matmul output region per PSUM bank
        # (partition-splits may share; column-splits may not).
        scorep = ctx.enter_context(
            tc.tile_pool(name="scorep", bufs=4, space="PSUM"))  # 4x1 bank
        pvp = ctx.enter_context(
            tc.tile_pool(name="pvp", bufs=2, space="PSUM"))     # 2x1 bank
        miscp = ctx.enter_context(
            tc.tile_pool(name="miscp", bufs=2, space="PSUM"))   # 2x1 bank

        ptp = ctx.enter_context(tc.tile_pool(name="ptp", bufs=40))
        zsbp = ctx.enter_context(tc.tile_pool(name="zsbp", bufs=2))

        # ---- PE p-state warmup ------------------------------------------
        # ~30 throwaway [128,128] matmuls run during the initial DMA wait so
        # the PE clock ramps to full speed before real work arrives.
        wps = miscp.tile([128, 128], f32, tag="misc", name="wps")
        for _ in range(30):
            nc.tensor.matmul(wps[:], wrm[:], wrm[:], start=True, stop=True,
                             skip_group_check=True)

        # ---- projection helpers ----------------------------------------
        def proj_pair(xhalves, w_sb, b_sb, dst, nb):
            # both m accumulation groups of a seq block, d-outer so the
            # d-step stream chases the input DMA arrivals; psum evacuated
            # (with bias) on ACT
            xh, off = nb // 2, (nb % 2) * 512
            ps = [miscp.tile([128, 512], f32, tag="misc", name="pps")
                  for _ in range(2)]
            for d in range(N_DT):
                for m in range(2):
                    nc.tensor.matmul(
                        ps[m][:], w_sb[:, d, m * 128:(m + 1) * 128],
                        xhalves[d][xh][:, off:off + 512],
                        start=(d == 0), stop=(d == N_DT - 1))
            for m in range(2):
                nc.scalar.activation(
                    dst[m][:, nb * 512:(nb + 1) * 512], ps[m][:],
                    AF.Identity, bias=b_sb[:, m:m + 1])

        def proj_v_st(st):
            # V projection, direct: out [s(128), e(256)] for s-tile st;
            # psum evacuated (with bias) alternating DVE/Pool
            xh = st // 8
            ps = miscp.tile([128, 512], f32, tag="misc", name="vps")
            for d in range(N_DT):
                nc.tensor.matmul(
                    ps[:, 0:E],
                    vts[d][xh][:, (st % 8) * 128:(st % 8) * 128 + 128],
                    wv_sb[:, d, :], start=(d == 0), stop=(d == N_DT - 1))
            eng = nc.vector if st % 2 == 0 else nc.gpsimd
            eng.tensor_tensor(
                vh65v[:, st, :, 0:DK],
                ps[:, 0:E].rearrange("p (h j) -> p h j", h=HPC),
                bvb_sb.rearrange("p (h j) -> p h j", h=HPC),
                op=mybir.AluOpType.add)

        # ---- per-slot emission pieces ----------------------------------
        zsb = {}
        pts_of = {}   # chunk -> list of pt tiles (one per kt)
        pv_of = {}    # (chunk, sweep) -> pv psum tile [128, 65]
        oc_of = {}    # chunk-pair -> oc tile [128, 4, 256]
        otT_of = {}   # chunk-pair -> otT tile [128, 2, 4, 128]
        exp_idx = [0]

        def emit_scores_exp_kt(c, kt):
            # two heads COOPERATE in one [128, 512] PSUM bank: the first
            # matmul (start=True) zeroes the whole 2KB zero-region, the
            # second accumulates into the untouched half (PE is in-order so
            # the pair cannot race); ONE exp call then drains the full bank.
            # Halves both the per-bank recycle chain (8 tiles/bank/step) and
            # the exp call count.  exp assigned by table across ACT/DVE/Pool.
            pt = ptp.tile([128, HPC * NQ], fp16, tag="pt", name="pt")
            for hp in range(2):
                sc = scorep.tile([128, 2 * NQ], f32, tag="sc", name="sc")
                for j in range(2):
                    nc.tensor.matmul(
                        sc[:, j * NQ:(j + 1) * NQ],
                        kh[hp][j * 64:(j + 1) * 64, kt * 128:(kt + 1) * 128],
                        qh[hp][j * 64:(j + 1) * 64, c * NQ:(c + 1) * NQ],
                        start=(j == 0), stop=(j == 1), skip_group_check=True)
                dst = pt[:, hp * 2 * NQ:(hp + 1) * 2 * NQ]
                eng = EXP_TABLE[exp_idx[0] % 32]
                exp_idx[0] += 1
                if eng == "A":
                    nc.scalar.activation(dst, sc[:], AF.Exp, scale=0.125,
                                         bias=ebias[:])
                else:
                    veng = nc.vector if eng == "D" else nc.gpsimd
                    veng.tensor_scalar(
                        dst.bitcast(i16), sc[:], SCHRA_A, SCHRA_B,
                        op0=ALU.mult, op1=ALU.add)
            pts_of.setdefault(c, []).append(pt)

        def emit_pv_half(c, s, half):
            # P@V sweep for (qb = s//4, head = s%4): 8 accumulating
            # matmuls (k-tiles half*8..half*8+7) into one [128, 65] bank
            qb, h = s // 4, s % 4
            if half == 0:
                pv_of[(c, s)] = pvp.tile([128, EV], f32, tag="pv", name="pv")
            pv = pv_of[(c, s)]
            pts = pts_of[c]
            for kt in range(half * 8, half * 8 + 8):
                nc.tensor.matmul(
                    pv[:],
                    pts[kt][:, h * NQ + qb * 128:h * NQ + qb * 128 + 128],
                    vh65v[:, kt, h, :],
                    start=(kt == 0), stop=(kt == N_KT - 1),
                    skip_group_check=True)

        def emit_norm_s(c, s):
            # ONE tensor_scalar divide by the denominator column, written
            # straight into the XBAR-transpose-ready oc layout
            qb, h = s // 4, s % 4
            pv = pv_of.pop((c, s))
            cp, qslot = c // 2, (c % 2) * 2 + qb
            if cp not in oc_of:
                oc_of[cp] = ocp.tile([128, 4, E], fp16, tag="oc", name="oc")
            dst = oc_of[cp][:, qslot, h * DK:(h + 1) * DK]
            eng = nc.vector if (c + s) % 2 == 0 else nc.gpsimd
            eng.tensor_scalar(dst, pv[:, 0:DK], pv[:, DK:DK + 1], None,
                              op0=ALU.divide)

        def emit_otT_dma(c, qb):
            # [q, e] -> [e, q] via the DMA XBAR: one call transposes both
            # 128-wide e-halves of (c, qb) into otT[:, :, qslot, :]
            cp, qslot = c // 2, (c % 2) * 2 + qb
            if cp not in otT_of:
                otT_of[cp] = otTp.tile([128, 2, 4, 128], fp16, tag="otT",
                                       name="otT")
            nc.sync.dma_start(otT_of[cp][:, :, qslot, :],
                              oc_of[cp][:, qslot, :], transpose=True)
            if qb == 1:
                pts_of.pop(c, None)
                if c % 2 == 1:
                    oc_of.pop(cp, None)

        def emit_outproj(cp, eo, qh):
            # out_proj partial for chunk-pair cp, qslot-half qh (256 q):
            # zT[eo-block, q-half].  qh0 (the even chunk's qslots) runs a
            # full 12 slots earlier than qh1 so the PE stays fed while the
            # odd chunk's P@V/norm/otT chain drains.  Evac spread over
            # ACT/DVE/Pool; zT DMA'd per 2-eo piece to shorten the tail.
            otT = otT_of[cp]
            zps = miscp.tile([128, 256], f32, tag="misc", name="zps")
            for cc in range(2):
                nc.tensor.matmul(
                    zps[:], wo_sb[:, cc, eo * 128:(eo + 1) * 128],
                    otT[:, cc, 2 * qh:2 * qh + 2, :],
                    start=(cc == 0), stop=(cc == 1), skip_group_check=True)
            if eo == 0 and qh == 0:
                zsb[cp] = zsbp.tile([128, 8, 2 * NQ], fp16, tag="zsb",
                                    name="zs")
            dst = zsb[cp][:, eo, qh * 256:(qh + 1) * 256]
            sel = (eo * 2 + qh) % 4
            if sel in (0, 2):
                nc.scalar.activation(dst, zps[:], AF.Copy)
            elif sel == 1:
                nc.vector.tensor_copy(dst, zps[:])
            else:
                nc.gpsimd.tensor_copy(dst, zps[:])
            if qh == 1 and eo % 2 == 1:
                nc.sync.dma_start(
                    zT.rearrange("(eo p) s -> p eo s", p=128)[
                        :, eo - 1:eo + 1, cp * 2 * NQ:(cp + 1) * 2 * NQ],
                    zsb[cp][:, eo - 1:eo + 1, :])
            if eo == 7 and qh == 1:
                otT_of.pop(cp, None)

        # ---- software pipeline -----------------------------------------
        # lead-in: k half0 projection + first q block
        proj_pair(kts, wk_sb, bk_sb, kh, 0)
        proj_pair(qts, wq_sb, bq_sb, qh, 0)
        proj_pair(kts, wk_sb, bk_sb, kh, 1)

        extras = {}

        def add_extra(cs, kt, fn):
            extras.setdefault((cs, kt), []).append(fn)

        # chunk0: k half1 at kt4/6 (needed by scores kt8+),
        #         V st0-7 at kt8-15, V st8-15 at step1 kt0-7 (needed by
        #         the PV(c0) sweeps which start at step1 slot 8)
        add_extra(0, 4, lambda: proj_pair(kts, wk_sb, bk_sb, kh, 2))
        add_extra(0, 6, lambda: proj_pair(kts, wk_sb, bk_sb, kh, 3))
        for i in range(8):
            add_extra(0, 8 + i, lambda st=i: proj_v_st(st))
        for i in range(8):
            add_extra(1, i, lambda st=8 + i: proj_v_st(st))
        add_extra(1, 8, lambda: proj_pair(qts, wq_sb, bq_sb, qh, 1))
        add_extra(2, 5, lambda: proj_pair(qts, wq_sb, bq_sb, qh, 2))
        add_extra(3, 5, lambda: proj_pair(qts, wq_sb, bq_sb, qh, 3))

        # Schedule per step/slot.  Sweeps of chunk c: qb0 (s 0-3) at step
        # c+1 slots 8-15, qb1 (s 4-7) at step c+2 slots 0-7; norms trail
        # each sweep; otT XBAR-transpose DMAs at step c+2 slots 1/9;
        # out_proj per chunk-pair at the odd chunk's step+2, slots 12-15.
        for step in range(N_QC + 2):
            for kt in range(N_KT):
                # qb1 sweeps + trailing norms for chunk step-2
                c2 = step - 2
                if 0 <= c2 <= N_QC - 1:
                    if kt < 8:
                        emit_pv_half(c2, 4 + kt // 2, kt % 2)
                    if kt == 0:
                        emit_norm_s(c2, 3)
                    if kt in (2, 4, 6, 8) and kt // 2 + 3 <= 7:
                        emit_norm_s(c2, kt // 2 + 3)
                    if kt == 1:
                        emit_otT_dma(c2, 0)
                    if kt == 9:
                        emit_otT_dma(c2, 1)
                    if c2 % 2 == 1 and kt <= 3:
                        emit_outproj(c2 // 2, 2 * kt, 0)
                        emit_outproj(c2 // 2, 2 * kt + 1, 0)
                    if c2 % 2 == 1 and 12 <= kt <= 15:
                        emit_outproj(c2 // 2, 2 * (kt - 12), 1)
                        emit_outproj(c2 // 2, 2 * (kt - 12) + 1, 1)
                # qb0 sweeps + norms for chunk step-1
                c1 = step - 1
                if 0 <= c1 <= N_QC - 1 and kt >= 8:
                    emit_pv_half(c1, (kt - 8) // 2, kt % 2)
                    if kt in (10, 12, 14):
                        emit_norm_s(c1, (kt - 10) // 2)
                for fn in extras.get((step, kt), ()):
                    fn()
                if step <= N_QC - 1:
                    emit_scores_exp_kt(step, kt)

    nc.compile()
    return nc


def _get_program():
    global _PROGRAM
    if _PROGRAM is None:
        _PROGRAM = _build_program()
    return _PROGRAM


def _make_in_maps(q, k, v, Wq, bq, Wk, bk, Wv, bv, Wo):
    f32 = np.float32
    xT = {}
    for b in range(B):
        xT[("q", b)] = np.ascontiguousarray(q[b].T, dtype=np.float16)
        xT[("k", b)] = np.ascontiguousarray(k[b].T, dtype=np.float16)
        xT[("v", b)] = np.ascontiguousarray(v[b].T, dtype=np.float16)
    wslices = {}
    for g in range(4):
        sl = slice(g * E, (g + 1) * E)
        wslices[("wq", g)] = np.ascontiguousarray(Wq[sl, :].T, dtype=np.float16)
        wslices[("wk", g)] = np.ascontiguousarray(Wk[sl, :].T, dtype=np.float16)
        wslices[("wv", g)] = np.ascontiguousarray(Wv[sl, :].T, dtype=np.float16)
        wslices[("wo", g)] = np.ascontiguousarray(Wo[:, sl].T, dtype=np.float16)
        wslices[("bq", g)] = np.ascontiguousarray(bq[sl].reshape(E, 1), dtype=f32)
        wslices[("bk", g)] = np.ascontiguousarray(bk[sl].reshape(E, 1), dtype=f32)
        wslices[("bvb", g)] = np.ascontiguousarray(
            np.tile(bv[sl].reshape(1, E), (128, 1)), dtype=f32)
    in_maps = []
    for c in range(N_CORES):
        b, g = c // 4, c % 4
        in_maps.append({
            "qT": xT[("q", b)], "kT": xT[("k", b)], "vT": xT[("v", b)],
            "wq": wslices[("wq", g)], "wk": wslices[("wk", g)],
            "wv": wslices[("wv", g)], "wo": wslices[("wo", g)],
            "bq": wslices[("bq", g)], "bk": wslices[("bk", g)],
            "bvb": wslices[("bvb", g)],
        })
    return in_maps


def _numpy_fallback(q, k, v, mask, Wq, bq, Wk, bk, Wv, bv, Wo, bo):
    # Only used if mask is not all-True (never the case for this problem).
    def proj(x, W, b_):
        y = x @ W.T + b_
        return y.reshape(B, S, NUM_HEADS, DK).transpose(0, 2, 1, 3)
    qh, kh, vh = proj(q, Wq, bq), proj(k, Wk, bk), proj(v, Wv, bv)
    sc = np.einsum("bhqd,bhkd->bhqk", qh, kh) / np.sqrt(DK)
    sc = np.where(mask, sc, np.float32(-1e9))
    sc = sc - sc.max(-1, keepdims=True)
    p = np.exp(sc)
    p /= p.sum(-1, keepdims=True)
    o = np.einsum("bhqk,bhkd->bhqd", p, vh)
    o = o.transpose(0, 2, 1, 3).reshape(B, S, D_MODEL)
    return (o @ Wo.T + bo).astype(np.float32)


def kernel(q, k, v, mask, Wq, bq, Wk, bk, Wv, bv, Wo, bo):
    q = np.asarray(q, dtype=np.float32)
    k = np.asarray(k, dtype=np.float32)
    v = np.asarray(v, dtype=np.float32)
    Wq, Wk, Wv, Wo = (np.asarray(w, dtype=np.float32) for w in (Wq, Wk, Wv, Wo))
    bq, bk, bv, bo = (np.asarray(x, dtype=np.float32) for x in (bq, bk, bv, bo))
    if not np.all(np.asarray(mask)):
        return _numpy_fallback(q, k, v, np.asarray(mask), Wq, bq, Wk, bk,
                               Wv, bv, Wo, bo)

    from concourse.bass_utils import run_bass_kernel_spmd
    nc = _get_program()
    in_maps = _make_in_maps(q, k, v, Wq, bq, Wk, bk, Wv, bv, Wo)
    res = run_bass_kernel_spmd(nc, in_maps, core_ids=list(range(N_CORES)),
                               **_RUN_KWARGS)
    global _LAST_RESULTS
    _LAST_RESULTS = res
    out = np.empty((B, S, D_MODEL), dtype=np.float32)
    for b in range(B):
        acc = res.results[4 * b]["zT"].astype(np.float32)
        for g in range(1, 4):
            acc = acc + res.results[4 * b + g]["zT"].astype(np.float32)
        out[b] = acc.T + bo
    return out


# revision 16
# speedup vs baseline: 1.0961x; 1.0449x over previous
"""Trainium2 Bass kernel for nn_MultiHeadAttention (B=2, S=2048, d_model=1024, H=16).

Sharding (8 cores): data-parallel over B (2) x tensor-parallel over head groups
(4 groups of 4 heads).  Each core computes its head-group's Q/K/V projections
(column-sharded weights), attention for its 4 heads, and a row-parallel
out_proj partial product.  The host sums the 4 partials per batch (the
"all-reduce") and adds the output bias.

Cost-model-driven design (matmul time = out-free-dim cycles, independent of
M/K; Ldweights engine-free):
  - scores computed transposed S_T[k, q] = Kh^T@Qh (kh stationary), exp'd in
    [128, 256] tiles split across THREE engines (ACT exact / DVE + Pool
    Schraudolph bit-trick) so no single engine's exp backlog stalls the
    PSUM-bank recycle that feeds the PE,
  - P@V uses the P tile as the STATIONARY operand and V as moving, with a
    ones-column appended to V: out[q, 65] per head -- column 64 is the
    softmax denominator for free,
  - normalization = ONE per-partition tensor_scalar divide (q on partitions),
    alternating DVE/Pool,
  - [q, e] -> [e, q] for the out_proj via the DMA XBAR transpose (14ns/tile
    on the DMA engines) instead of PE transpose matmuls + copies,
  - V projection computed directly non-transposed (vT tile stationary, Wv
    moving) -- no V transposes,
  - PE warmed up with throwaway matmuls during the initial DMA wait so the
    p-state ramp completes before real work arrives,
  - fp16 operands everywhere (fp32 PSUM accumulation).
"""

import sys
import numpy as np

for _p in ("/opt/trn_rl_repo", "/root/.axon_site/_ro/trn_rl_repo"):
    if _p not in sys.path:
        sys.path.append(_p)

D_MODEL = 1024
NUM_HEADS = 16
DK = 64
B = 2
S = 2048
N_CORES = 8
HPC = 4               # heads per core
E = HPC * DK          # 256 features per core
NQ = 256              # q-chunk size
N_QC = S // NQ        # 8 q chunks
N_KT = S // 128       # 16 k tiles
N_DT = D_MODEL // 128  # 8 contraction tiles for projections
EV = DK + 1           # V feature block incl. ones column (denominator)

_PROGRAM = None
_RUN_KWARGS = {}      # test harness may set {"trace": True}
_LAST_RESULTS = None  # BassKernelResults of the last run


def _build_program():
    import concourse.bass as bass
    import concourse.mybir as mybir
    from concourse import bacc, tile
    from contextlib import ExitStack

    f32 = mybir.dt.float32
    fp16 = mybir.dt.float16
    i16 = mybir.dt.int16
    AF = mybir.ActivationFunctionType
    ALU = mybir.AluOpType

    # exp split: ACT computes exact exp (with a bias that matches the
    # Schraudolph C1 factor); DVE and Pool compute a Schraudolph-style
    # bit-trick exp in ONE op (i16 = round(a*s + b), read back by the P@V
    # matmul through a bitcast-to-fp16 AP).  The bit pattern evaluates
    # C1*exp(s/8) with C1 = 1.04085 (mean mantissa-interpolation factor,
    # +-3% deviation); softmax cancels the common C1 row-wise.
    EXP_BIAS = float(np.log(1.0408461))
    SCHRA_A = 0.125 * 1024.0 / float(np.log(2.0))
    SCHRA_B = 15.0 * 1024.0

    # engine assignment table for exp tiles: 12 ACT / 12 DVE / 8 Pool per 32
    _t = list("ADP" * 11)[:32]
    _t[29] = "A"
    _t[26] = "D"
    EXP_TABLE = "".join(_t)

    nc = bacc.Bacc("TRN2", target_bir_lowering=False, debug=False,
                   num_devices=N_CORES)

    # Per-core DRAM I/O (transposed activations, pre-sliced weights)
    qT = nc.dram_tensor("qT", [D_MODEL, S], fp16, kind="ExternalInput").ap()
    kT = nc.dram_tensor("kT", [D_MODEL, S], fp16, kind="ExternalInput").ap()
    vT = nc.dram_tensor("vT", [D_MODEL, S], fp16, kind="ExternalInput").ap()
    wq = nc.dram_tensor("wq", [D_MODEL, E], fp16, kind="ExternalInput").ap()
    wk = nc.dram_tensor("wk", [D_MODEL, E], fp16, kind="ExternalInput").ap()
    wv = nc.dram_tensor("wv", [D_MODEL, E], fp16, kind="ExternalInput").ap()
    wo = nc.dram_tensor("wo", [E, D_MODEL], fp16, kind="ExternalInput").ap()
    bq = nc.dram_tensor("bq", [E, 1], f32, kind="ExternalInput").ap()
    bk = nc.dram_tensor("bk", [E, 1], f32, kind="ExternalInput").ap()
    bvb = nc.dram_tensor("bvb", [128, E], f32, kind="ExternalInput").ap()
    zT = nc.dram_tensor("zT", [D_MODEL, S], fp16, kind="ExternalOutput").ap()

    with tile.TileContext(nc) as tc, ExitStack() as ctx:
        persist = ctx.enter_context(tc.tile_pool(name="persist", bufs=1))
        const = ctx.enter_context(tc.tile_pool(name="const", bufs=1))

        # ---- weights + biases resident in SBUF --------------------------
        wq_sb = persist.tile([128, N_DT, E], fp16, tag="wq", name="wq")
        wk_sb = persist.tile([128, N_DT, E], fp16, tag="wk", name="wk")
        wv_sb = persist.tile([128, N_DT, E], fp16, tag="wv", name="wv")
        wo_sb = persist.tile([128, 2, D_MODEL], fp16, tag="wo", name="wo")
        bq_sb = persist.tile([128, 2], f32, tag="bq", name="bq")
        bk_sb = persist.tile([128, 2], f32, tag="bk", name="bk")
        bvb_sb = persist.tile([128, E], f32, tag="bvb", name="bvb")

        # PE warmup tile: all-zero fp16 operand for throwaway ramp matmuls
        wrm = const.tile([128, 128], fp16, tag="wrm", name="wrm")
        nc.gpsimd.memset(wrm[:], 0.0)

        # input stream tiles: [128, 2, 1024] half-pairs (two d-tiles per DMA)
        xpool = ctx.enter_context(tc.tile_pool(name="xpool", bufs=12))

        def load_half_pair(src, d, h):
            # two d-tiles in one DMA: halves the HWDGE issue cost on the
            # startup-critical k/q half-0 stream
            xt = xpool.tile([128, 2, S // 2], fp16, tag="xt2", name="xt2")
            nc.sync.dma_start(
                xt[:], src.rearrange("(t p) s -> p t s", p=128)[
                    :, d:d + 2, h * (S // 2):(h + 1) * (S // 2)])
            return [xt[:, 0, :], xt[:, 1, :]]

        # startup-critical loads first: wk d0 slice, then kT half0 in 2d
        # pairs chased by the lead-in projection, then wq + qT half0 ...
        wkv = wk.rearrange("(t p) e -> p t e", p=128)
        kts = [[None, None] for _ in range(N_DT)]
        qts = [[None, None] for _ in range(N_DT)]
        vts = [[None, None] for _ in range(N_DT)]

        nc.sync.dma_start(wk_sb[:, 0:1, :], wkv[:, 0:1, :])
        kts[0][0], kts[1][0] = load_half_pair(kT, 0, 0)
        nc.sync.dma_start(wk_sb[:, 1:4, :], wkv[:, 1:4, :])
        kts[2][0], kts[3][0] = load_half_pair(kT, 2, 0)
        nc.sync.dma_start(wk_sb[:, 4:8, :], wkv[:, 4:8, :])
        nc.sync.dma_start(bk_sb[:], bk.rearrange("(m p) o -> p (m o)", p=128))
        kts[4][0], kts[5][0] = load_half_pair(kT, 4, 0)
        kts[6][0], kts[7][0] = load_half_pair(kT, 6, 0)
        nc.sync.dma_start(wq_sb[:], wq.rearrange("(t p) e -> p t e", p=128))
        nc.sync.dma_start(bq_sb[:], bq.rearrange("(m p) o -> p (m o)", p=128))
        qts[0][0], qts[1][0] = load_half_pair(qT, 0, 0)
        qts[2][0], qts[3][0] = load_half_pair(qT, 2, 0)
        qts[4][0], qts[5][0] = load_half_pair(qT, 4, 0)
        qts[6][0], qts[7][0] = load_half_pair(qT, 6, 0)
        for d in range(0, N_DT, 2):
            kts[d][1], kts[d + 1][1] = load_half_pair(kT, d, 1)
        nc.sync.dma_start(wv_sb[:], wv.rearrange("(t p) e -> p t e", p=128))
        nc.sync.dma_start(bvb_sb[:], bvb)
        for d in range(0, N_DT, 2):
            vts[d][0], vts[d + 1][0] = load_half_pair(vT, d, 0)
        for d in range(0, N_DT, 2):
            vts[d][1], vts[d + 1][1] = load_half_pair(vT, d, 1)
        for d in range(0, N_DT, 2):
            qts[d][1], qts[d + 1][1] = load_half_pair(qT, d, 1)
        nc.sync.dma_start(wo_sb[:], wo.rearrange("(t p) e -> p t e", p=128))

        ebias = const.tile([128, 1], f32, tag="ebias", name="ebias")
        nc.gpsimd.memset(ebias[:], EXP_BIAS)

        # ---- persistent activations ------------------------------------
        qh = [persist.tile([128, S], fp16, tag=f"qh{p}", name=f"qh{p}")
              for p in range(2)]
        kh = [persist.tile([128, S], fp16, tag=f"kh{p}", name=f"kh{p}")
              for p in range(2)]
        # V projection non-transposed: [s(128-tile), kt, head, 65]
        vh65 = persist.tile([128, N_KT * HPC * EV], fp16, tag="vh65",
                            name="vh65")
        vh65v = vh65.rearrange("p (t h c) -> p t h c", t=N_KT, h=HPC)
        # ones columns for the softmax denominators
        nc.gpsimd.memset(vh65v[:, :, :, DK:DK + 1], 1.0)

        # normalized attention out per chunk-pair, laid out for the XBAR
        # transpose: [q(128), qslot(4), e(256)]
        ocp = ctx.enter_context(tc.tile_pool(name="ocp", bufs=2))
        # transposed: [e(128), cc(2), qslot(4), q(128)]
        otTp = ctx.enter_context(tc.tile_pool(name="otTp", bufs=2))

        # ---- PSUM pools (exactly 8 banks) ------------------------------
        # HW rule: one (non-transpose) matmul output region per PSUM bank
        # (partition-splits may share; column-splits may not).
        scorep = ctx.enter_context(
            tc.tile_pool(name="scorep", bufs=4, space="PSUM"))  # 4x1 bank
        pvp = ctx.enter_context(
            tc.tile_pool(name="pvp", bufs=2, space="PSUM"))     # 2x1 bank
        miscp = ctx.enter_context(
            tc.tile_pool(name="miscp", bufs=2, space="PSUM"))   # 2x1 bank

        ptp = ctx.enter_context(tc.tile_pool(name="ptp", bufs=36))
        zsbp = ctx.enter_context(tc.tile_pool(name="zsbp", bufs=2))

        # ---- PE p-state warmup ------------------------------------------
        # ~30 throwaway [128,128] matmuls run during the initial DMA wait so
        # the PE clock ramps to full speed before real work arrives.
        wps = miscp.tile([128, 128], f32, tag="misc", name="wps")
        for _ in range(30):
            nc.tensor.matmul(wps[:], wrm[:], wrm[:], start=True, stop=True,
                             skip_group_check=True)

        # ---- projection helpers ----------------------------------------
        def proj_pair(xhalves, w_sb, b_sb, dst, nb):
            # both m accumulation groups of a seq block, d-outer so the
            # d-step stream chases the input DMA arrivals; psum evacuated
            # (with bias) on ACT
            xh, off = nb // 2, (nb % 2) * 512
            ps = [miscp.tile([128, 512], f32, tag="misc", name="pps")
                  for _ in range(2)]
            for d in range(N_DT):
                for m in range(2):
                    nc.tensor.matmul(
                        ps[m][:], w_sb[:, d, m * 128:(m + 1) * 128],
                        xhalves[d][xh][:, off:off + 512],
                        start=(d == 0), stop=(d == N_DT - 1))
            for m in range(2):
                nc.scalar.activation(
                    dst[m][:, nb * 512:(nb + 1) * 512], ps[m][:],
                    AF.Identity, bias=b_sb[:, m:m + 1])

        def proj_v_st(st):
            # V projection, direct: out [s(128), e(256)] for s-tile st;
            # psum evacuated (with bias) alternating DVE/Pool
            xh = st // 8
            ps = miscp.tile([128, 512], f32, tag="misc", name="vps")
            for d in range(N_DT):
                nc.tensor.matmul(
                    ps[:, 0:E],
                    vts[d][xh][:, (st % 8) * 128:(st % 8) * 128 + 128],
                    wv_sb[:, d, :], start=(d == 0), stop=(d == N_DT - 1))
            eng = nc.vector if st % 2 == 0 else nc.gpsimd
            eng.tensor_tensor(
                vh65v[:, st, :, 0:DK],
                ps[:, 0:E].rearrange("p (h j) -> p h j", h=HPC),
                bvb_sb.rearrange("p (h j) -> p h j", h=HPC),
                op=mybir.AluOpType.add)

        # ---- per-slot emission pieces ----------------------------------
        zsb = {}
        pts_of = {}   # chunk -> list of pt tiles (one per kt)
        pv_of = {}    # (chunk, sweep) -> pv psum tile [128, 65]
        oc_of = {}    # chunk-pair -> oc tile [128, 4, 256]
        otT_of = {}   # chunk-pair -> otT tile [128, 2, 4, 128]
        exp_idx = [0]

        def emit_scores_exp_kt(c, kt):
            # two heads COOPERATE in one [128, 512] PSUM bank: the first
            # matmul (start=True) zeroes the whole 2KB zero-region, the
            # second accumulates into the untouched half (PE is in-order so
            # the pair cannot race); ONE exp call then drains the full bank.
            # Halves both the per-bank recycle chain (8 tiles/bank/step) and
            # the exp call count.  exp assigned by table across ACT/DVE/Pool.
            pt = ptp.tile([128, HPC * NQ], fp16, tag="pt", name="pt")
            for hp in range(2):
                sc = scorep.tile([128, 2 * NQ], f32, tag="sc", name="sc")
                for j in range(2):
                    nc.tensor.matmul(
                        sc[:, j * NQ:(j + 1) * NQ],
                        kh[hp][j * 64:(j + 1) * 64, kt * 128:(kt + 1) * 128],
                        qh[hp][j * 64:(j + 1) * 64, c * NQ:(c + 1) * NQ],
                        start=(j == 0), stop=(j == 1), skip_group_check=True)
                dst = pt[:, hp * 2 * NQ:(hp + 1) * 2 * NQ]
                eng = EXP_TABLE[exp_idx[0] % 32]
                exp_idx[0] += 1
                if eng == "A":
                    nc.scalar.activation(dst, sc[:], AF.Exp, scale=0.125,
                                         bias=ebias[:])
                else:
                    veng = nc.vector if eng == "D" else nc.gpsimd
                    veng.tensor_scalar(
                        dst.bitcast(i16), sc[:], SCHRA_A, SCHRA_B,
                        op0=ALU.mult, op1=ALU.add)
            pts_of.setdefault(c, []).append(pt)

        def emit_pv_half(c, s, half):
            # P@V sweep for (qb = s//4, head = s%4): 8 accumulating
            # matmuls (k-tiles half*8..half*8+7) into one [128, 65] bank
            qb, h = s // 4, s % 4
            if half == 0:
                pv_of[(c, s)] = pvp.tile([128, EV], f32, tag="pv", name="pv")
            pv = pv_of[(c, s)]
            pts = pts_of[c]
            for kt in range(half * 8, half * 8 + 8):
                nc.tensor.matmul(
                    pv[:],
                    pts[kt][:, h * NQ + qb * 128:h * NQ + qb * 128 + 128],
                    vh65v[:, kt, h, :],
                    start=(kt == 0), stop=(kt == N_KT - 1),
                    skip_group_check=True)

        def emit_norm_s(c, s):
            # ONE tensor_scalar divide by the denominator column, written
            # straight into the XBAR-transpose-ready oc layout
            qb, h = s // 4, s % 4
            pv = pv_of.pop((c, s))
            cp, qslot = c // 2, (c % 2) * 2 + qb
            if cp not in oc_of:
                oc_of[cp] = ocp.tile([128, 4, E], fp16, tag="oc", name="oc")
            dst = oc_of[cp][:, qslot, h * DK:(h + 1) * DK]
            eng = nc.vector if (c + s) % 2 == 0 else nc.gpsimd
            eng.tensor_scalar(dst, pv[:, 0:DK], pv[:, DK:DK + 1], None,
                              op0=ALU.divide)

        def emit_otT_dma(c, qb):
            # [q, e] -> [e, q] via the DMA XBAR: one call transposes both
            # 128-wide e-halves of (c, qb) into otT[:, :, qslot, :]
            cp, qslot = c // 2, (c % 2) * 2 + qb
            if cp not in otT_of:
                otT_of[cp] = otTp.tile([128, 2, 4, 128], fp16, tag="otT",
                                       name="otT")
            nc.sync.dma_start(otT_of[cp][:, :, qslot, :],
                              oc_of[cp][:, qslot, :], transpose=True)
            if qb == 1 and c % 2 == 1:
                oc_of.pop(cp, None)

        def emit_outproj(cp, eo, qh):
            # out_proj partial for chunk-pair cp, qslot-half qh (256 q):
            # zT[eo-block, q-half].  qh0 (the even chunk's qslots) runs a
            # full 12 slots earlier than qh1 so the PE stays fed while the
            # odd chunk's P@V/norm/otT chain drains.  Evac spread over
            # ACT/DVE/Pool; zT DMA'd per 2-eo piece to shorten the tail.
            otT = otT_of[cp]
            zps = miscp.tile([128, 256], f32, tag="misc", name="zps")
            for cc in range(2):
                nc.tensor.matmul(
                    zps[:], wo_sb[:, cc, eo * 128:(eo + 1) * 128],
                    otT[:, cc, 2 * qh:2 * qh + 2, :],
                    start=(cc == 0), stop=(cc == 1), skip_group_check=True)
            if eo == 0 and qh == 0:
                zsb[cp] = zsbp.tile([128, 8, 2 * NQ], fp16, tag="zsb",
                                    name="zs")
            dst = zsb[cp][:, eo, qh * 256:(qh + 1) * 256]
            sel = (eo * 2 + qh) % 4
            if sel in (0, 2):
                nc.scalar.activation(dst, zps[:], AF.Copy)
            elif sel == 1:
                nc.vector.tensor_copy(dst, zps[:])
            else:
                nc.gpsimd.tensor_copy(dst, zps[:])
            if qh == 1 and eo % 2 == 1:
                nc.sync.dma_start(
                    zT.rearrange("(eo p) s -> p eo s", p=128)[
                        :, eo - 1:eo + 1, cp * 2 * NQ:(cp + 1) * 2 * NQ],
                    zsb[cp][:, eo - 1:eo + 1, :])
            if eo == 7 and qh == 1:
                otT_of.pop(cp, None)

        # ---- software pipeline -----------------------------------------
        # lead-in: k half0 projection (both col-blocks, k DMAs land first),
        # then the first q block (PE is in-order, so k-dep work goes first)
        proj_pair(kts, wk_sb, bk_sb, kh, 0)
        proj_pair(kts, wk_sb, bk_sb, kh, 1)
        proj_pair(qts, wq_sb, bq_sb, qh, 0)

        extras = {}

        def add_extra(cs, kt, fn):
            extras.setdefault((cs, kt), []).append(fn)

        # chunk0: k half1 at kt4/6 (needed by scores kt8+),
        #         V st0-7 at kt8-15, V st8-15 at step1 kt0-7 (needed by
        #         the PV(c0) sweeps which start at step1 slot 8)
        add_extra(0, 4, lambda: proj_pair(kts, wk_sb, bk_sb, kh, 2))
        add_extra(0, 6, lambda: proj_pair(kts, wk_sb, bk_sb, kh, 3))
        for i in range(8):
            add_extra(0, 8 + i, lambda st=i: proj_v_st(st))
        for i in range(8):
            add_extra(1, i, lambda st=8 + i: proj_v_st(st))
        add_extra(1, 8, lambda: proj_pair(qts, wq_sb, bq_sb, qh, 1))
        add_extra(2, 5, lambda: proj_pair(qts, wq_sb, bq_sb, qh, 2))
        add_extra(3, 5, lambda: proj_pair(qts, wq_sb, bq_sb, qh, 3))

        # Schedule per step/slot.  ALL sweeps of chunk c run at step c+1
        # slots 8-15 (sweep s = kt-8, both halves); norms trail one slot
        # (s 0-6 at kt 9-15, s 7 at step c+2 kt 0); otT XBAR-transpose DMAs
        # at step c+2 slots 1/2; out_proj per chunk-pair at the odd chunk's
        # step+2, slots 6-13 (after the otT DMA latency), zT DMA'd per
        # 2-eo piece as rows complete.
        for step in range(N_QC + 2):
            for kt in range(N_KT):
                # trailing norm + otT + out_proj for chunk step-2
                c2 = step - 2
                if 0 <= c2 <= N_QC - 1:
                    if kt == 0:
                        emit_norm_s(c2, 7)
                        pts_of.pop(c2, None)
                    if kt == 1:
                        emit_otT_dma(c2, 0)
                    if kt == 2:
                        emit_otT_dma(c2, 1)
                    if c2 % 2 == 1 and 6 <= kt <= 13:
                        emit_outproj(c2 // 2, kt - 6, 0)
                        emit_outproj(c2 // 2, kt - 6, 1)
                # sweeps + trailing norms for chunk step-1
                c1 = step - 1
                if 0 <= c1 <= N_QC - 1:
                    if kt >= 8:
                        emit_pv_half(c1, kt - 8, 0)
                        emit_pv_half(c1, kt - 8, 1)
                    if kt >= 9:
                        emit_norm_s(c1, kt - 9)
                for fn in extras.get((step, kt), ()):
                    fn()
                if step <= N_QC - 1:
                    emit_scores_exp_kt(step, kt)

    nc.compile()
    return nc


def _get_program():
    global _PROGRAM
    if _PROGRAM is None:
        _PROGRAM = _build_program()
    return _PROGRAM


def _make_in_maps(q, k, v, Wq, bq, Wk, bk, Wv, bv, Wo):
    f32 = np.float32
    xT = {}
    for b in range(B):
        xT[("q", b)] = np.ascontiguousarray(q[b].T, dtype=np.float16)
        xT[("k", b)] = np.ascontiguousarray(k[b].T, dtype=np.float16)
        xT[("v", b)] = np.ascontiguousarray(v[b].T, dtype=np.float16)
    wslices = {}
    for g in range(4):
        sl = slice(g * E, (g + 1) * E)
        wslices[("wq", g)] = np.ascontiguousarray(Wq[sl, :].T, dtype=np.float16)
        wslices[("wk", g)] = np.ascontiguousarray(Wk[sl, :].T, dtype=np.float16)
        wslices[("wv", g)] = np.ascontiguousarray(Wv[sl, :].T, dtype=np.float16)
        wslices[("wo", g)] = np.ascontiguousarray(Wo[:, sl].T, dtype=np.float16)
        wslices[("bq", g)] = np.ascontiguousarray(bq[sl].reshape(E, 1), dtype=f32)
        wslices[("bk", g)] = np.ascontiguousarray(bk[sl].reshape(E, 1), dtype=f32)
        wslices[("bvb", g)] = np.ascontiguousarray(
            np.tile(bv[sl].reshape(1, E), (128, 1)), dtype=f32)
    in_maps = []
    for c in range(N_CORES):
        b, g = c // 4, c % 4
        in_maps.append({
            "qT": xT[("q", b)], "kT": xT[("k", b)], "vT": xT[("v", b)],
            "wq": wslices[("wq", g)], "wk": wslices[("wk", g)],
            "wv": wslices[("wv", g)], "wo": wslices[("wo", g)],
            "bq": wslices[("bq", g)], "bk": wslices[("bk", g)],
            "bvb": wslices[("bvb", g)],
        })
    return in_maps


def _numpy_fallback(q, k, v, mask, Wq, bq, Wk, bk, Wv, bv, Wo, bo):
    # Only used if mask is not all-True (never the case for this problem).
    def proj(x, W, b_):
        y = x @ W.T + b_
        return y.reshape(B, S, NUM_HEADS, DK).transpose(0, 2, 1, 3)
    qh, kh, vh = proj(q, Wq, bq), proj(k, Wk, bk), proj(v, Wv, bv)
    sc = np.einsum("bhqd,bhkd->bhqk", qh, kh) / np.sqrt(DK)
    sc = np.where(mask, sc, np.float32(-1e9))
    sc = sc - sc.max(-1, keepdims=True)
    p = np.exp(sc)
    p /= p.sum(-1, keepdims=True)
    o = np.einsum("bhqk,bhkd->bhqd", p, vh)
    o = o.transpose(0, 2, 1, 3).reshape(B, S, D_MODEL)
    return (o @ Wo.T + bo).astype(np.float32)


def kernel(q, k, v, mask, Wq, bq, Wk, bk, Wv, bv, Wo, bo):
    q = np.asarray(q, dtype=np.float32)
    k = np.asarray(k, dtype=np.float32)
    v = np.asarray(v, dtype=np.float32)
    Wq, Wk, Wv, Wo = (np.asarray(w, dtype=np.float32) for w in (Wq, Wk, Wv, Wo))
    bq, bk, bv, bo = (np.asarray(x, dtype=np.float32) for x in (bq, bk, bv, bo))
    if not np.all(np.asarray(mask)):
        return _numpy_fallback(q, k, v, np.asarray(mask), Wq, bq, Wk, bk,
                               Wv, bv, Wo, bo)

    from concourse.bass_utils import run_bass_kernel_spmd
    nc = _get_program()
    in_maps = _make_in_maps(q, k, v, Wq, bq, Wk, bk, Wv, bv, Wo)
    res = run_bass_kernel_spmd(nc, in_maps, core_ids=list(range(N_CORES)),
                               **_RUN_KWARGS)
    global _LAST_RESULTS
    _LAST_RESULTS = res
    out = np.empty((B, S, D_MODEL), dtype=np.float32)
    for b in range(B):
        acc = res.results[4 * b]["zT"].astype(np.float32)
        for g in range(1, 4):
            acc = acc + res.results[4 * b + g]["zT"].astype(np.float32)
        out[b] = acc.T + bo
    return out


# revision 20
# speedup vs baseline: 1.0971x; 1.0009x over previous
"""Trainium2 Bass kernel for nn_MultiHeadAttention (B=2, S=2048, d_model=1024, H=16).

Sharding (8 cores): data-parallel over B (2) x tensor-parallel over head groups
(4 groups of 4 heads).  Each core computes its head-group's Q/K/V projections
(column-sharded weights), attention for its 4 heads, and a row-parallel
out_proj partial product.  The host sums the 4 partials per batch (the
"all-reduce") and adds the output bias.

Cost-model-driven design (matmul time = out-free-dim cycles, independent of
M/K; Ldweights engine-free):
  - scores computed transposed S_T[k, q] = Kh^T@Qh (kh stationary), exp'd in
    [128, 256] tiles split across THREE engines (ACT exact / DVE + Pool
    Schraudolph bit-trick) so no single engine's exp backlog stalls the
    PSUM-bank recycle that feeds the PE,
  - P@V uses the P tile as the STATIONARY operand and V as moving, with a
    ones-column appended to V: out[q, 65] per head -- column 64 is the
    softmax denominator for free,
  - normalization = ONE per-partition tensor_scalar divide (q on partitions),
    alternating DVE/Pool,
  - [q, e] -> [e, q] for the out_proj via the DMA XBAR transpose (14ns/tile
    on the DMA engines) instead of PE transpose matmuls + copies,
  - V projection computed directly non-transposed (vT tile stationary, Wv
    moving) -- no V transposes,
  - PE warmed up with throwaway matmuls during the initial DMA wait so the
    p-state ramp completes before real work arrives,
  - fp16 operands everywhere (fp32 PSUM accumulation).
"""

import sys
import numpy as np

for _p in ("/opt/trn_rl_repo", "/root/.axon_site/_ro/trn_rl_repo"):
    if _p not in sys.path:
        sys.path.append(_p)

D_MODEL = 1024
NUM_HEADS = 16
DK = 64
B = 2
S = 2048
N_CORES = 8
HPC = 4               # heads per core
E = HPC * DK          # 256 features per core
NQ = 256              # q-chunk size
N_QC = S // NQ        # 8 q chunks
N_KT = S // 128       # 16 k tiles
N_DT = D_MODEL // 128  # 8 contraction tiles for projections
EV = DK + 1           # V feature block incl. ones column (denominator)

_PROGRAM = None
_RUN_KWARGS = {}      # test harness may set {"trace": True}
_LAST_RESULTS = None  # BassKernelResults of the last run


def _build_program():
    import concourse.bass as bass
    import concourse.mybir as mybir
    from concourse import bacc, tile
    from contextlib import ExitStack

    f32 = mybir.dt.float32
    fp16 = mybir.dt.float16
    i16 = mybir.dt.int16
    AF = mybir.ActivationFunctionType
    ALU = mybir.AluOpType

    # exp split: ACT computes exact exp (with a bias that matches the
    # Schraudolph C1 factor); DVE and Pool compute a Schraudolph-style
    # bit-trick exp in ONE op (i16 = round(a*s + b), read back by the P@V
    # matmul through a bitcast-to-fp16 AP).  The bit pattern evaluates
    # C1*exp(s/8) with C1 = 1.04085 (mean mantissa-interpolation factor,
    # +-3% deviation); softmax cancels the common C1 row-wise.
    EXP_BIAS = float(np.log(1.0408461))
    SCHRA_A = 0.125 * 1024.0 / float(np.log(2.0))
    SCHRA_B = 15.0 * 1024.0

    # engine assignment for exp tiles: the last kts of each step go to the
    # fast low-latency engines (ACT/DVE) so the next step's score banks
    # recycle quickly; the rest cycle a 26-entry table (9 ACT/9 DVE/8 Pool)
    EXP_TABLE = "ADP" * 8 + "AD"

    nc = bacc.Bacc("TRN2", target_bir_lowering=False, debug=False,
                   num_devices=N_CORES)

    # Per-core DRAM I/O (transposed activations, pre-sliced weights)
    qT = nc.dram_tensor("qT", [D_MODEL, S], fp16, kind="ExternalInput").ap()
    kT = nc.dram_tensor("kT", [D_MODEL, S], fp16, kind="ExternalInput").ap()
    vT = nc.dram_tensor("vT", [D_MODEL, S], fp16, kind="ExternalInput").ap()
    wq = nc.dram_tensor("wq", [D_MODEL, E], fp16, kind="ExternalInput").ap()
    wk = nc.dram_tensor("wk", [D_MODEL, E], fp16, kind="ExternalInput").ap()
    wv = nc.dram_tensor("wv", [D_MODEL, E], fp16, kind="ExternalInput").ap()
    wo = nc.dram_tensor("wo", [E, D_MODEL], fp16, kind="ExternalInput").ap()
    bq = nc.dram_tensor("bq", [E, 1], f32, kind="ExternalInput").ap()
    bk = nc.dram_tensor("bk", [E, 1], f32, kind="ExternalInput").ap()
    bvb = nc.dram_tensor("bvb", [128, E], f32, kind="ExternalInput").ap()
    zT = nc.dram_tensor("zT", [D_MODEL, S], fp16, kind="ExternalOutput").ap()

    with tile.TileContext(nc) as tc, ExitStack() as ctx:
        persist = ctx.enter_context(tc.tile_pool(name="persist", bufs=1))
        const = ctx.enter_context(tc.tile_pool(name="const", bufs=1))

        # ---- weights + biases resident in SBUF --------------------------
        wq_sb = persist.tile([128, N_DT, E], fp16, tag="wq", name="wq")
        wk_sb = persist.tile([128, N_DT, E], fp16, tag="wk", name="wk")
        wv_sb = persist.tile([128, N_DT, E], fp16, tag="wv", name="wv")
        wo_sb = persist.tile([128, 2, D_MODEL], fp16, tag="wo", name="wo")
        bq_sb = persist.tile([128, 2], f32, tag="bq", name="bq")
        bk_sb = persist.tile([128, 2], f32, tag="bk", name="bk")
        bvb_sb = persist.tile([128, E], f32, tag="bvb", name="bvb")

        # PE warmup tile: all-zero fp16 operand for throwaway ramp matmuls
        wrm = const.tile([128, 128], fp16, tag="wrm", name="wrm")
        nc.gpsimd.memset(wrm[:], 0.0)

        # input stream tiles: [128, 2, 1024] half-pairs (two d-tiles per DMA)
        xpool = ctx.enter_context(tc.tile_pool(name="xpool", bufs=12))

        def load_half_pair(src, d, h):
            # two d-tiles in one DMA: halves the HWDGE issue cost on the
            # startup-critical k/q half-0 stream
            xt = xpool.tile([128, 2, S // 2], fp16, tag="xt2", name="xt2")
            nc.sync.dma_start(
                xt[:], src.rearrange("(t p) s -> p t s", p=128)[
                    :, d:d + 2, h * (S // 2):(h + 1) * (S // 2)])
            return [xt[:, 0, :], xt[:, 1, :]]

        # startup-critical loads first: wk d0 slice, then kT half0 in 2d
        # pairs chased by the lead-in projection, then wq + qT half0 ...
        wkv = wk.rearrange("(t p) e -> p t e", p=128)
        kts = [[None, None] for _ in range(N_DT)]
        qts = [[None, None] for _ in range(N_DT)]
        vts = [[None, None] for _ in range(N_DT)]

        nc.sync.dma_start(wk_sb[:, 0:1, :], wkv[:, 0:1, :])
        kts[0][0], kts[1][0] = load_half_pair(kT, 0, 0)
        nc.sync.dma_start(wk_sb[:, 1:4, :], wkv[:, 1:4, :])
        kts[2][0], kts[3][0] = load_half_pair(kT, 2, 0)
        nc.sync.dma_start(wk_sb[:, 4:8, :], wkv[:, 4:8, :])
        nc.sync.dma_start(bk_sb[:], bk.rearrange("(m p) o -> p (m o)", p=128))
        kts[4][0], kts[5][0] = load_half_pair(kT, 4, 0)
        kts[6][0], kts[7][0] = load_half_pair(kT, 6, 0)
        nc.sync.dma_start(wq_sb[:], wq.rearrange("(t p) e -> p t e", p=128))
        nc.sync.dma_start(bq_sb[:], bq.rearrange("(m p) o -> p (m o)", p=128))
        qts[0][0], qts[1][0] = load_half_pair(qT, 0, 0)
        qts[2][0], qts[3][0] = load_half_pair(qT, 2, 0)
        qts[4][0], qts[5][0] = load_half_pair(qT, 4, 0)
        qts[6][0], qts[7][0] = load_half_pair(qT, 6, 0)
        for d in range(0, N_DT, 2):
            kts[d][1], kts[d + 1][1] = load_half_pair(kT, d, 1)
        nc.sync.dma_start(wv_sb[:], wv.rearrange("(t p) e -> p t e", p=128))
        nc.sync.dma_start(bvb_sb[:], bvb)
        for d in range(0, N_DT, 2):
            vts[d][0], vts[d + 1][0] = load_half_pair(vT, d, 0)
        for d in range(0, N_DT, 2):
            vts[d][1], vts[d + 1][1] = load_half_pair(vT, d, 1)
        for d in range(0, N_DT, 2):
            qts[d][1], qts[d + 1][1] = load_half_pair(qT, d, 1)
        nc.sync.dma_start(wo_sb[:], wo.rearrange("(t p) e -> p t e", p=128))

        ebias = const.tile([128, 1], f32, tag="ebias", name="ebias")
        nc.gpsimd.memset(ebias[:], EXP_BIAS)

        # ---- persistent activations ------------------------------------
        qh = [persist.tile([128, S], fp16, tag=f"qh{p}", name=f"qh{p}")
              for p in range(2)]
        kh = [persist.tile([128, S], fp16, tag=f"kh{p}", name=f"kh{p}")
              for p in range(2)]
        # V projection non-transposed: [s(128-tile), kt, head, 65]
        vh65 = persist.tile([128, N_KT * HPC * EV], fp16, tag="vh65",
                            name="vh65")
        vh65v = vh65.rearrange("p (t h c) -> p t h c", t=N_KT, h=HPC)
        # ones columns for the softmax denominators
        nc.gpsimd.memset(vh65v[:, :, :, DK:DK + 1], 1.0)

        # normalized attention out per chunk-pair, laid out for the XBAR
        # transpose: [q(128), qslot(4), e(256)]
        ocp = ctx.enter_context(tc.tile_pool(name="ocp", bufs=2))
        # transposed: [e(128), cc(2), qslot(4), q(128)]
        otTp = ctx.enter_context(tc.tile_pool(name="otTp", bufs=2))

        # ---- PSUM pools (exactly 8 banks) ------------------------------
        # HW rule: one (non-transpose) matmul output region per PSUM bank
        # (partition-splits may share; column-splits may not).
        scorep = ctx.enter_context(
            tc.tile_pool(name="scorep", bufs=4, space="PSUM"))  # 4x1 bank
        pvp = ctx.enter_context(
            tc.tile_pool(name="pvp", bufs=2, space="PSUM"))     # 2x1 bank
        miscp = ctx.enter_context(
            tc.tile_pool(name="miscp", bufs=2, space="PSUM"))   # 2x1 bank

        ptp = ctx.enter_context(tc.tile_pool(name="ptp", bufs=36))
        zsbp = ctx.enter_context(tc.tile_pool(name="zsbp", bufs=2))

        # ---- PE p-state warmup ------------------------------------------
        # ~30 throwaway [128,128] matmuls run during the initial DMA wait so
        # the PE clock ramps to full speed before real work arrives.
        wps = miscp.tile([128, 128], f32, tag="misc", name="wps")
        for _ in range(30):
            nc.tensor.matmul(wps[:], wrm[:], wrm[:], start=True, stop=True,
                             skip_group_check=True)

        # ---- projection helpers ----------------------------------------
        def proj_pair(xhalves, w_sb, b_sb, dst, nb):
            # both m accumulation groups of a seq block, d-outer so the
            # d-step stream chases the input DMA arrivals; psum evacuated
            # (with bias) on ACT
            xh, off = nb // 2, (nb % 2) * 512
            ps = [miscp.tile([128, 512], f32, tag="misc", name="pps")
                  for _ in range(2)]
            for d in range(N_DT):
                for m in range(2):
                    nc.tensor.matmul(
                        ps[m][:], w_sb[:, d, m * 128:(m + 1) * 128],
                        xhalves[d][xh][:, off:off + 512],
                        start=(d == 0), stop=(d == N_DT - 1))
            for m in range(2):
                nc.scalar.activation(
                    dst[m][:, nb * 512:(nb + 1) * 512], ps[m][:],
                    AF.Identity, bias=b_sb[:, m:m + 1])

        def proj_v_st(st):
            # V projection, direct: out [s(128), e(256)] for s-tile st;
            # psum evacuated (with bias) alternating DVE/Pool
            xh = st // 8
            ps = miscp.tile([128, 512], f32, tag="misc", name="vps")
            for d in range(N_DT):
                nc.tensor.matmul(
                    ps[:, 0:E],
                    vts[d][xh][:, (st % 8) * 128:(st % 8) * 128 + 128],
                    wv_sb[:, d, :], start=(d == 0), stop=(d == N_DT - 1))
            eng = nc.vector if st % 2 == 0 else nc.gpsimd
            eng.tensor_tensor(
                vh65v[:, st, :, 0:DK],
                ps[:, 0:E].rearrange("p (h j) -> p h j", h=HPC),
                bvb_sb.rearrange("p (h j) -> p h j", h=HPC),
                op=mybir.AluOpType.add)

        # ---- per-slot emission pieces ----------------------------------
        zsb = {}
        pts_of = {}   # chunk -> list of pt tiles (one per kt)
        pv_of = {}    # (chunk, sweep) -> pv psum tile [128, 65]
        oc_of = {}    # chunk-pair -> oc tile [128, 4, 256]
        otT_of = {}   # chunk-pair -> otT tile [128, 2, 4, 128]
        exp_idx = [0]

        def emit_scores_exp_kt(c, kt):
            # two heads COOPERATE in one [128, 512] PSUM bank: the first
            # matmul (start=True) zeroes the whole 2KB zero-region, the
            # second accumulates into the untouched half (PE is in-order so
            # the pair cannot race); ONE exp call then drains the full bank.
            # Halves both the per-bank recycle chain (8 tiles/bank/step) and
            # the exp call count.  exp assigned by table across ACT/DVE/Pool.
            pt = ptp.tile([128, HPC * NQ], fp16, tag="pt", name="pt")
            for hp in range(2):
                sc = scorep.tile([128, 2 * NQ], f32, tag="sc", name="sc")
                for j in range(2):
                    nc.tensor.matmul(
                        sc[:, j * NQ:(j + 1) * NQ],
                        kh[hp][j * 64:(j + 1) * 64, kt * 128:(kt + 1) * 128],
                        qh[hp][j * 64:(j + 1) * 64, c * NQ:(c + 1) * NQ],
                        start=(j == 0), stop=(j == 1), skip_group_check=True)
                dst = pt[:, hp * 2 * NQ:(hp + 1) * 2 * NQ]
                if kt >= 13:
                    eng = "A" if (kt + hp) % 2 == 0 else "D"
                else:
                    eng = EXP_TABLE[exp_idx[0] % 26]
                    exp_idx[0] += 1
                if eng == "A":
                    nc.scalar.activation(dst, sc[:], AF.Exp, scale=0.125,
                                         bias=ebias[:])
                else:
                    veng = nc.vector if eng == "D" else nc.gpsimd
                    veng.tensor_scalar(
                        dst.bitcast(i16), sc[:], SCHRA_A, SCHRA_B,
                        op0=ALU.mult, op1=ALU.add)
            pts_of.setdefault(c, []).append(pt)

        def emit_pv_half(c, s, half):
            # P@V sweep for (qb = s//4, head = s%4): 8 accumulating
            # matmuls (k-tiles half*8..half*8+7) into one [128, 65] bank
            qb, h = s // 4, s % 4
            if half == 0:
                pv_of[(c, s)] = pvp.tile([128, EV], f32, tag="pv", name="pv")
            pv = pv_of[(c, s)]
            pts = pts_of[c]
            for kt in range(half * 8, half * 8 + 8):
                nc.tensor.matmul(
                    pv[:],
                    pts[kt][:, h * NQ + qb * 128:h * NQ + qb * 128 + 128],
                    vh65v[:, kt, h, :],
                    start=(kt == 0), stop=(kt == N_KT - 1),
                    skip_group_check=True)

        def emit_norm_s(c, s):
            # ONE tensor_scalar divide by the denominator column, written
            # straight into the XBAR-transpose-ready oc layout
            qb, h = s // 4, s % 4
            pv = pv_of.pop((c, s))
            cp, qslot = c // 2, (c % 2) * 2 + qb
            if cp not in oc_of:
                oc_of[cp] = ocp.tile([128, 4, E], fp16, tag="oc", name="oc")
            dst = oc_of[cp][:, qslot, h * DK:(h + 1) * DK]
            eng = nc.vector if (c + s) % 2 == 0 else nc.gpsimd
            eng.tensor_scalar(dst, pv[:, 0:DK], pv[:, DK:DK + 1], None,
                              op0=ALU.divide)

        def emit_otT_dma(c, qb):
            # [q, e] -> [e, q] via the DMA XBAR: one call transposes both
            # 128-wide e-halves of (c, qb) into otT[:, :, qslot, :]
            cp, qslot = c // 2, (c % 2) * 2 + qb
            if cp not in otT_of:
                otT_of[cp] = otTp.tile([128, 2, 4, 128], fp16, tag="otT",
                                       name="otT")
            nc.sync.dma_start(otT_of[cp][:, :, qslot, :],
                              oc_of[cp][:, qslot, :], transpose=True)
            if qb == 1 and c % 2 == 1:
                oc_of.pop(cp, None)

        def emit_outproj(cp, eo, qh):
            # out_proj partial for chunk-pair cp, qslot-half qh (256 q):
            # zT[eo-block, q-half].  qh0 (the even chunk's qslots) runs a
            # full 12 slots earlier than qh1 so the PE stays fed while the
            # odd chunk's P@V/norm/otT chain drains.  Evac spread over
            # ACT/DVE/Pool; zT DMA'd per 2-eo piece to shorten the tail.
            otT = otT_of[cp]
            zps = miscp.tile([128, 256], f32, tag="misc", name="zps")
            for cc in range(2):
                nc.tensor.matmul(
                    zps[:], wo_sb[:, cc, eo * 128:(eo + 1) * 128],
                    otT[:, cc, 2 * qh:2 * qh + 2, :],
                    start=(cc == 0), stop=(cc == 1), skip_group_check=True)
            if eo == 0 and qh == 0:
                zsb[cp] = zsbp.tile([128, 8, 2 * NQ], fp16, tag="zsb",
                                    name="zs")
            dst = zsb[cp][:, eo, qh * 256:(qh + 1) * 256]
            sel = (eo + 2 * qh) % 4
            if sel in (0, 2):
                nc.scalar.activation(dst, zps[:], AF.Copy)
            elif sel == 1:
                nc.vector.tensor_copy(dst, zps[:])
            else:
                nc.gpsimd.tensor_copy(dst, zps[:])
            if qh == 1 and eo % 2 == 1:
                nc.sync.dma_start(
                    zT.rearrange("(eo p) s -> p eo s", p=128)[
                        :, eo - 1:eo + 1, cp * 2 * NQ:(cp + 1) * 2 * NQ],
                    zsb[cp][:, eo - 1:eo + 1, :])
            if eo == 7 and qh == 1:
                otT_of.pop(cp, None)

        # ---- software pipeline -----------------------------------------
        # lead-in: k half0 projection (both col-blocks, k DMAs land first),
        # then the first q block (PE is in-order, so k-dep work goes first)
        proj_pair(kts, wk_sb, bk_sb, kh, 0)
        proj_pair(kts, wk_sb, bk_sb, kh, 1)
        proj_pair(qts, wq_sb, bq_sb, qh, 0)

        extras = {}

        def add_extra(cs, kt, fn):
            extras.setdefault((cs, kt), []).append(fn)

        # chunk0: k half1 at kt4/6 (needed by scores kt8+),
        #         V st0-7 at kt8-15, V st8-15 at step1 kt0-7 (needed by
        #         the PV(c0) sweeps which start at step1 slot 8)
        add_extra(0, 4, lambda: proj_pair(kts, wk_sb, bk_sb, kh, 2))
        add_extra(0, 6, lambda: proj_pair(kts, wk_sb, bk_sb, kh, 3))
        for i in range(8):
            add_extra(0, 8 + i, lambda st=i: proj_v_st(st))
        for i in range(8):
            add_extra(1, i, lambda st=8 + i: proj_v_st(st))
        add_extra(1, 8, lambda: proj_pair(qts, wq_sb, bq_sb, qh, 1))
        add_extra(2, 5, lambda: proj_pair(qts, wq_sb, bq_sb, qh, 2))
        add_extra(3, 5, lambda: proj_pair(qts, wq_sb, bq_sb, qh, 3))

        # Schedule per step/slot.  ALL sweeps of chunk c run at step c+1
        # slots 8-15 (sweep s = kt-8, both halves); norms trail one slot
        # (s 0-6 at kt 9-15, s 7 at step c+2 kt 0); otT XBAR-transpose DMAs
        # at step c+2 slots 1/2; out_proj per chunk-pair at the odd chunk's
        # step+2, slots 6-13 (after the otT DMA latency), zT DMA'd per
        # 2-eo piece as rows complete.
        for step in range(N_QC + 2):
            for kt in range(N_KT):
                # trailing norm + otT + out_proj for chunk step-2
                c2 = step - 2
                if 0 <= c2 <= N_QC - 1:
                    if kt == 0:
                        emit_norm_s(c2, 7)
                        pts_of.pop(c2, None)
                    if kt == 1:
                        emit_otT_dma(c2, 1)
                    if c2 % 2 == 1:
                        if kt <= 3:
                            emit_outproj(c2 // 2, 2 * kt, 0)
                            emit_outproj(c2 // 2, 2 * kt + 1, 0)
                        if 6 <= kt <= 13:
                            emit_outproj(c2 // 2, kt - 6, 1)
                # sweeps + trailing norms for chunk step-1
                c1 = step - 1
                if 0 <= c1 <= N_QC - 1:
                    if kt >= 8:
                        emit_pv_half(c1, kt - 8, 0)
                        emit_pv_half(c1, kt - 8, 1)
                    if kt >= 9:
                        emit_norm_s(c1, kt - 9)
                    if kt == 14:
                        emit_otT_dma(c1, 0)
                for fn in extras.get((step, kt), ()):
                    fn()
                if step <= N_QC - 1:
                    emit_scores_exp_kt(step, kt)

    nc.compile()
    return nc


def _get_program():
    global _PROGRAM
    if _PROGRAM is None:
        _PROGRAM = _build_program()
    return _PROGRAM


def _make_in_maps(q, k, v, Wq, bq, Wk, bk, Wv, bv, Wo):
    f32 = np.float32
    xT = {}
    for b in range(B):
        xT[("q", b)] = np.ascontiguousarray(q[b].T, dtype=np.float16)
        xT[("k", b)] = np.ascontiguousarray(k[b].T, dtype=np.float16)
        xT[("v", b)] = np.ascontiguousarray(v[b].T, dtype=np.float16)
    wslices = {}
    for g in range(4):
        sl = slice(g * E, (g + 1) * E)
        wslices[("wq", g)] = np.ascontiguousarray(Wq[sl, :].T, dtype=np.float16)
        wslices[("wk", g)] = np.ascontiguousarray(Wk[sl, :].T, dtype=np.float16)
        wslices[("wv", g)] = np.ascontiguousarray(Wv[sl, :].T, dtype=np.float16)
        wslices[("wo", g)] = np.ascontiguousarray(Wo[:, sl].T, dtype=np.float16)
        wslices[("bq", g)] = np.ascontiguousarray(bq[sl].reshape(E, 1), dtype=f32)
        wslices[("bk", g)] = np.ascontiguousarray(bk[sl].reshape(E, 1), dtype=f32)
        wslices[("bvb", g)] = np.ascontiguousarray(
            np.tile(bv[sl].reshape(1, E), (128, 1)), dtype=f32)
    in_maps = []
    for c in range(N_CORES):
        b, g = c // 4, c % 4
        in_maps.append({
            "qT": xT[("q", b)], "kT": xT[("k", b)], "vT": xT[("v", b)],
            "wq": wslices[("wq", g)], "wk": wslices[("wk", g)],
            "wv": wslices[("wv", g)], "wo": wslices[("wo", g)],
            "bq": wslices[("bq", g)], "bk": wslices[("bk", g)],
            "bvb": wslices[("bvb", g)],
        })
    return in_maps


def _numpy_fallback(q, k, v, mask, Wq, bq, Wk, bk, Wv, bv, Wo, bo):
    # Only used if mask is not all-True (never the case for this problem).
    def proj(x, W, b_):
        y = x @ W.T + b_
        return y.reshape(B, S, NUM_HEADS, DK).transpose(0, 2, 1, 3)
    qh, kh, vh = proj(q, Wq, bq), proj(k, Wk, bk), proj(v, Wv, bv)
    sc = np.einsum("bhqd,bhkd->bhqk", qh, kh) / np.sqrt(DK)
    sc = np.where(mask, sc, np.float32(-1e9))
    sc = sc - sc.max(-1, keepdims=True)
    p = np.exp(sc)
    p /= p.sum(-1, keepdims=True)
    o = np.einsum("bhqk,bhkd->bhqd", p, vh)
    o = o.transpose(0, 2, 1, 3).reshape(B, S, D_MODEL)
    return (o @ Wo.T + bo).astype(np.float32)


def kernel(q, k, v, mask, Wq, bq, Wk, bk, Wv, bv, Wo, bo):
    q = np.asarray(q, dtype=np.float32)
    k = np.asarray(k, dtype=np.float32)
    v = np.asarray(v, dtype=np.float32)
    Wq, Wk, Wv, Wo = (np.asarray(w, dtype=np.float32) for w in (Wq, Wk, Wv, Wo))
    bq, bk, bv, bo = (np.asarray(x, dtype=np.float32) for x in (bq, bk, bv, bo))
    if not np.all(np.asarray(mask)):
        return _numpy_fallback(q, k, v, np.asarray(mask), Wq, bq, Wk, bk,
                               Wv, bv, Wo, bo)

    from concourse.bass_utils import run_bass_kernel_spmd
    nc = _get_program()
    in_maps = _make_in_maps(q, k, v, Wq, bq, Wk, bk, Wv, bv, Wo)
    res = run_bass_kernel_spmd(nc, in_maps, core_ids=list(range(N_CORES)),
                               **_RUN_KWARGS)
    global _LAST_RESULTS
    _LAST_RESULTS = res
    out = np.empty((B, S, D_MODEL), dtype=np.float32)
    for b in range(B):
        acc = res.results[4 * b]["zT"].astype(np.float32)
        for g in range(1, 4):
            acc = acc + res.results[4 * b + g]["zT"].astype(np.float32)
        out[b] = acc.T + bo
    return out


# revision 25
# speedup vs baseline: 1.1075x; 1.0095x over previous
"""Trainium2 Bass kernel for nn_MultiHeadAttention (B=2, S=2048, d_model=1024, H=16).

Sharding (8 cores): data-parallel over B (2) x tensor-parallel over head groups
(4 groups of 4 heads).  Each core computes its head-group's Q/K/V projections
(column-sharded weights), attention for its 4 heads, and a row-parallel
out_proj partial product.  The host sums the 4 partials per batch (the
"all-reduce") and adds the output bias.

Cost-model-driven design (matmul time = out-free-dim cycles, independent of
M/K; Ldweights engine-free):
  - scores computed transposed S_T[k, q] = Kh^T@Qh (kh stationary), exp'd in
    [128, 256] tiles split across THREE engines (ACT exact / DVE + Pool
    Schraudolph bit-trick) so no single engine's exp backlog stalls the
    PSUM-bank recycle that feeds the PE,
  - P@V uses the P tile as the STATIONARY operand and V as moving, with a
    ones-column appended to V: out[q, 65] per head -- column 64 is the
    softmax denominator for free,
  - normalization = ONE per-partition tensor_scalar divide (q on partitions),
    alternating DVE/Pool,
  - [q, e] -> [e, q] for the out_proj via the DMA XBAR transpose (14ns/tile
    on the DMA engines) instead of PE transpose matmuls + copies,
  - V projection computed directly non-transposed (vT tile stationary, Wv
    moving) -- no V transposes,
  - PE warmed up with throwaway matmuls during the initial DMA wait so the
    p-state ramp completes before real work arrives,
  - fp16 operands everywhere (fp32 PSUM accumulation).
"""

import sys
import numpy as np

for _p in ("/opt/trn_rl_repo", "/root/.axon_site/_ro/trn_rl_repo"):
    if _p not in sys.path:
        sys.path.append(_p)

D_MODEL = 1024
NUM_HEADS = 16
DK = 64
B = 2
S = 2048
N_CORES = 8
HPC = 4               # heads per core
E = HPC * DK          # 256 features per core
NQ = 256              # q-chunk size
N_QC = S // NQ        # 8 q chunks
N_KT = S // 128       # 16 k tiles
N_DT = D_MODEL // 128  # 8 contraction tiles for projections
EV = DK + 1           # V feature block incl. ones column (denominator)

_PROGRAM = None
_RUN_KWARGS = {}      # test harness may set {"trace": True}
_LAST_RESULTS = None  # BassKernelResults of the last run


def _build_program():
    import concourse.bass as bass
    import concourse.mybir as mybir
    from concourse import bacc, tile
    from contextlib import ExitStack

    f32 = mybir.dt.float32
    fp16 = mybir.dt.float16
    i16 = mybir.dt.int16
    AF = mybir.ActivationFunctionType
    ALU = mybir.AluOpType

    # exp split: ACT computes exact exp (with a bias that matches the
    # Schraudolph C1 factor); DVE and Pool compute a Schraudolph-style
    # bit-trick exp in ONE op (i16 = round(a*s + b), read back by the P@V
    # matmul through a bitcast-to-fp16 AP).  The bit pattern evaluates
    # C1*exp(s/8) with C1 = 1.04085 (mean mantissa-interpolation factor,
    # +-3% deviation); softmax cancels the common C1 row-wise.
    EXP_BIAS = float(np.log(1.0408461))
    SCHRA_A = 0.125 * 1024.0 / float(np.log(2.0))
    SCHRA_B = 15.0 * 1024.0

    # engine assignment for exp tiles: the last kts of each step go to the
    # fast low-latency engines (ACT/DVE) so the next step's score banks
    # recycle quickly; the rest cycle a 26-entry table (9 ACT/9 DVE/8 Pool)
    EXP_TABLE = "ADP" * 8 + "AD"

    nc = bacc.Bacc("TRN2", target_bir_lowering=False, debug=False,
                   num_devices=N_CORES)

    # Per-core DRAM I/O (transposed activations, pre-sliced weights)
    qT = nc.dram_tensor("qT", [D_MODEL, S], fp16, kind="ExternalInput").ap()
    kT = nc.dram_tensor("kT", [D_MODEL, S], fp16, kind="ExternalInput").ap()
    vT = nc.dram_tensor("vT", [D_MODEL, S], fp16, kind="ExternalInput").ap()
    wq = nc.dram_tensor("wq", [D_MODEL, E], fp16, kind="ExternalInput").ap()
    wk = nc.dram_tensor("wk", [D_MODEL, E], fp16, kind="ExternalInput").ap()
    wv = nc.dram_tensor("wv", [D_MODEL, E], fp16, kind="ExternalInput").ap()
    wo = nc.dram_tensor("wo", [E, D_MODEL], fp16, kind="ExternalInput").ap()
    bq = nc.dram_tensor("bq", [E, 1], f32, kind="ExternalInput").ap()
    bk = nc.dram_tensor("bk", [E, 1], f32, kind="ExternalInput").ap()
    bvb = nc.dram_tensor("bvb", [128, E], f32, kind="ExternalInput").ap()
    zT = nc.dram_tensor("zT", [D_MODEL, S], fp16, kind="ExternalOutput").ap()

    with tile.TileContext(nc) as tc, ExitStack() as ctx:
        persist = ctx.enter_context(tc.tile_pool(name="persist", bufs=1))
        const = ctx.enter_context(tc.tile_pool(name="const", bufs=1))

        # ---- weights + biases resident in SBUF --------------------------
        wq_sb = persist.tile([128, N_DT, E], fp16, tag="wq", name="wq")
        wk_sb = persist.tile([128, N_DT, E], fp16, tag="wk", name="wk")
        wv_sb = persist.tile([128, N_DT, E], fp16, tag="wv", name="wv")
        wo_sb = persist.tile([128, 2, D_MODEL], fp16, tag="wo", name="wo")
        bq_sb = persist.tile([128, 2], f32, tag="bq", name="bq")
        bk_sb = persist.tile([128, 2], f32, tag="bk", name="bk")
        bvb_sb = persist.tile([128, E], f32, tag="bvb", name="bvb")

        # PE warmup tile: all-zero fp16 operand for throwaway ramp matmuls
        wrm = const.tile([128, 128], fp16, tag="wrm", name="wrm")
        nc.gpsimd.memset(wrm[:], 0.0)

        # input stream tiles: [128, 2, 1024] half-pairs (two d-tiles per DMA)
        xpool = ctx.enter_context(tc.tile_pool(name="xpool", bufs=12))

        def load_half_pair(src, d, h):
            # two d-tiles in one DMA: halves the HWDGE issue cost on the
            # startup-critical k/q half-0 stream
            xt = xpool.tile([128, 2, S // 2], fp16, tag="xt2", name="xt2")
            nc.sync.dma_start(
                xt[:], src.rearrange("(t p) s -> p t s", p=128)[
                    :, d:d + 2, h * (S // 2):(h + 1) * (S // 2)])
            return [xt[:, 0, :], xt[:, 1, :]]

        # startup-critical loads first: wk d0 slice, then kT half0 in 2d
        # pairs chased by the lead-in projection, then wq + qT half0 ...
        wkv = wk.rearrange("(t p) e -> p t e", p=128)
        kts = [[None, None] for _ in range(N_DT)]
        qts = [[None, None] for _ in range(N_DT)]
        vts = [[None, None] for _ in range(N_DT)]

        nc.sync.dma_start(wk_sb[:, 0:1, :], wkv[:, 0:1, :])
        kts[0][0], kts[1][0] = load_half_pair(kT, 0, 0)
        nc.sync.dma_start(wk_sb[:, 1:4, :], wkv[:, 1:4, :])
        kts[2][0], kts[3][0] = load_half_pair(kT, 2, 0)
        nc.sync.dma_start(wk_sb[:, 4:8, :], wkv[:, 4:8, :])
        nc.sync.dma_start(bk_sb[:], bk.rearrange("(m p) o -> p (m o)", p=128))
        kts[4][0], kts[5][0] = load_half_pair(kT, 4, 0)
        kts[6][0], kts[7][0] = load_half_pair(kT, 6, 0)
        nc.sync.dma_start(wq_sb[:], wq.rearrange("(t p) e -> p t e", p=128))
        nc.sync.dma_start(bq_sb[:], bq.rearrange("(m p) o -> p (m o)", p=128))
        qts[0][0], qts[1][0] = load_half_pair(qT, 0, 0)
        qts[2][0], qts[3][0] = load_half_pair(qT, 2, 0)
        qts[4][0], qts[5][0] = load_half_pair(qT, 4, 0)
        qts[6][0], qts[7][0] = load_half_pair(qT, 6, 0)
        for d in range(0, N_DT, 2):
            kts[d][1], kts[d + 1][1] = load_half_pair(kT, d, 1)
        nc.sync.dma_start(wv_sb[:], wv.rearrange("(t p) e -> p t e", p=128))
        nc.sync.dma_start(bvb_sb[:], bvb)
        for d in range(0, N_DT, 2):
            vts[d][0], vts[d + 1][0] = load_half_pair(vT, d, 0)
        for d in range(0, N_DT, 2):
            vts[d][1], vts[d + 1][1] = load_half_pair(vT, d, 1)
        for d in range(0, N_DT, 2):
            qts[d][1], qts[d + 1][1] = load_half_pair(qT, d, 1)
        nc.sync.dma_start(wo_sb[:], wo.rearrange("(t p) e -> p t e", p=128))

        ebias = const.tile([128, 1], f32, tag="ebias", name="ebias")
        nc.gpsimd.memset(ebias[:], EXP_BIAS)

        from concourse.masks import make_identity
        ident = const.tile([128, 128], fp16, tag="ident", name="ident")
        make_identity(nc, ident)

        # ---- persistent activations ------------------------------------
        qh = [persist.tile([128, S], fp16, tag=f"qh{p}", name=f"qh{p}")
              for p in range(2)]
        kh = [persist.tile([128, S], fp16, tag=f"kh{p}", name=f"kh{p}")
              for p in range(2)]
        # V projection non-transposed: [s(128-tile), kt, head, 65]
        vh65 = persist.tile([128, N_KT * HPC * EV], fp16, tag="vh65",
                            name="vh65")
        vh65v = vh65.rearrange("p (t h c) -> p t h c", t=N_KT, h=HPC)
        # ones columns for the softmax denominators
        nc.gpsimd.memset(vh65v[:, :, :, DK:DK + 1], 1.0)

        # normalized attention out per chunk-pair, laid out for the XBAR
        # transpose: [q(128), qslot(4), e(256)]
        ocp = ctx.enter_context(tc.tile_pool(name="ocp", bufs=2))
        # transposed: [e(128), cc(2), qslot(4), q(128)]
        otTp = ctx.enter_context(tc.tile_pool(name="otTp", bufs=2))

        # ---- PSUM pools (exactly 8 banks) ------------------------------
        # HW rule: one (non-transpose) matmul output region per PSUM bank
        # (partition-splits may share; column-splits may not).
        scorep = ctx.enter_context(
            tc.tile_pool(name="scorep", bufs=4, space="PSUM"))  # 4x1 bank
        pvp = ctx.enter_context(
            tc.tile_pool(name="pvp", bufs=2, space="PSUM"))     # 2x1 bank
        miscp = ctx.enter_context(
            tc.tile_pool(name="miscp", bufs=2, space="PSUM"))   # 2x1 bank

        ptp = ctx.enter_context(tc.tile_pool(name="ptp", bufs=36))
        zsbp = ctx.enter_context(tc.tile_pool(name="zsbp", bufs=2))

        # ---- PE p-state warmup ------------------------------------------
        # ~30 throwaway [128,128] matmuls run during the initial DMA wait so
        # the PE clock ramps to full speed before real work arrives.
        wps = miscp.tile([128, 128], f32, tag="misc", name="wps")
        for _ in range(30):
            nc.tensor.matmul(wps[:], wrm[:], wrm[:], start=True, stop=True,
                             skip_group_check=True)

        # ---- projection helpers ----------------------------------------
        def proj_pair(xhalves, w_sb, b_sb, dst, nb):
            # both m accumulation groups of a seq block, d-outer so the
            # d-step stream chases the input DMA arrivals; psum evacuated
            # (with bias) on ACT
            xh, off = nb // 2, (nb % 2) * 512
            ps = [miscp.tile([128, 512], f32, tag="misc", name="pps")
                  for _ in range(2)]
            for d in range(N_DT):
                for m in range(2):
                    nc.tensor.matmul(
                        ps[m][:], w_sb[:, d, m * 128:(m + 1) * 128],
                        xhalves[d][xh][:, off:off + 512],
                        start=(d == 0), stop=(d == N_DT - 1))
            for m in range(2):
                nc.scalar.activation(
                    dst[m][:, nb * 512:(nb + 1) * 512], ps[m][:],
                    AF.Identity, bias=b_sb[:, m:m + 1])

        def proj_v_st(st):
            # V projection, direct: out [s(128), e(256)] for s-tile st;
            # psum evacuated (with bias) alternating DVE/Pool
            xh = st // 8
            ps = miscp.tile([128, 512], f32, tag="misc", name="vps")
            for d in range(N_DT):
                nc.tensor.matmul(
                    ps[:, 0:E],
                    vts[d][xh][:, (st % 8) * 128:(st % 8) * 128 + 128],
                    wv_sb[:, d, :], start=(d == 0), stop=(d == N_DT - 1))
            eng = nc.vector if st % 2 == 0 else nc.gpsimd
            eng.tensor_tensor(
                vh65v[:, st, :, 0:DK],
                ps[:, 0:E].rearrange("p (h j) -> p h j", h=HPC),
                bvb_sb.rearrange("p (h j) -> p h j", h=HPC),
                op=mybir.AluOpType.add)

        # ---- per-slot emission pieces ----------------------------------
        zsb = {}
        pts_of = {}   # chunk -> list of pt tiles (one per kt)
        pv_of = {}    # (chunk, sweep) -> pv psum tile [128, 65]
        oc_of = {}    # chunk-pair -> oc tile [128, 4, 256]
        otT_of = {}   # chunk-pair -> otT tile [128, 2, 4, 128]
        exp_idx = [0]

        def emit_scores_exp_kt(c, kt):
            # two heads COOPERATE in one [128, 512] PSUM bank: the first
            # matmul (start=True) zeroes the whole 2KB zero-region, the
            # second accumulates into the untouched half (PE is in-order so
            # the pair cannot race); ONE exp call then drains the full bank.
            # Halves both the per-bank recycle chain (8 tiles/bank/step) and
            # the exp call count.  exp assigned by table across ACT/DVE/Pool.
            pt = ptp.tile([128, HPC * NQ], fp16, tag="pt", name="pt")
            for hp in range(2):
                sc = scorep.tile([128, 2 * NQ], f32, tag="sc", name="sc")
                for j in range(2):
                    nc.tensor.matmul(
                        sc[:, j * NQ:(j + 1) * NQ],
                        kh[hp][j * 64:(j + 1) * 64, kt * 128:(kt + 1) * 128],
                        qh[hp][j * 64:(j + 1) * 64, c * NQ:(c + 1) * NQ],
                        start=(j == 0), stop=(j == 1), skip_group_check=True)
                dst = pt[:, hp * 2 * NQ:(hp + 1) * 2 * NQ]
                if kt >= 13:
                    eng = "A" if (kt + hp) % 2 == 0 else "D"
                else:
                    eng = EXP_TABLE[exp_idx[0] % 26]
                    exp_idx[0] += 1
                if eng == "A":
                    nc.scalar.activation(dst, sc[:], AF.Exp, scale=0.125,
                                         bias=ebias[:])
                else:
                    veng = nc.vector if eng == "D" else nc.gpsimd
                    veng.tensor_scalar(
                        dst.bitcast(i16), sc[:], SCHRA_A, SCHRA_B,
                        op0=ALU.mult, op1=ALU.add)
            pts_of.setdefault(c, []).append(pt)

        def emit_pv_half(c, s, half):
            # P@V sweep for (qb = s//4, head = s%4): 8 accumulating
            # matmuls (k-tiles half*8..half*8+7) into one [128, 65] bank
            qb, h = s // 4, s % 4
            if half == 0:
                pv_of[(c, s)] = pvp.tile([128, EV], f32, tag="pv", name="pv")
            pv = pv_of[(c, s)]
            pts = pts_of[c]
            for kt in range(half * 8, half * 8 + 8):
                nc.tensor.matmul(
                    pv[:],
                    pts[kt][:, h * NQ + qb * 128:h * NQ + qb * 128 + 128],
                    vh65v[:, kt, h, :],
                    start=(kt == 0), stop=(kt == N_KT - 1),
                    skip_group_check=True)

        def emit_norm_s(c, s):
            # ONE tensor_scalar divide by the denominator column, written
            # straight into the XBAR-transpose-ready oc layout
            qb, h = s // 4, s % 4
            pv = pv_of.pop((c, s))
            cp, qslot = c // 2, (c % 2) * 2 + qb
            if cp not in oc_of:
                oc_of[cp] = ocp.tile([128, 4, E], fp16, tag="oc", name="oc")
            dst = oc_of[cp][:, qslot, h * DK:(h + 1) * DK]
            eng = nc.vector if (c + s) % 2 == 0 else nc.gpsimd
            eng.tensor_scalar(dst, pv[:, 0:DK], pv[:, DK:DK + 1], None,
                              op0=ALU.divide)

        def emit_otT_dma(c, qb):
            # [q, e] -> [e, q] via the DMA XBAR: one call transposes both
            # 128-wide e-halves of (c, qb) into otT[:, :, qslot, :].  The
            # last chunk transposes on the (by then idle) PE instead: the
            # 2.4us XBAR DMA latency would sit on the critical tail.
            cp, qslot = c // 2, (c % 2) * 2 + qb
            if cp not in otT_of:
                otT_of[cp] = otTp.tile([128, 2, 4, 128], fp16, tag="otT",
                                       name="otT")
            if c == N_QC - 1:
                tp = scorep.tile([128, 256], fp16, tag="sc", name="tp")
                for bb in range(2):
                    nc.tensor.matmul(
                        tp[:, bb * 128:(bb + 1) * 128],
                        oc_of[cp][:, qslot, bb * 128:(bb + 1) * 128],
                        ident[:], is_transpose=True, start=True, stop=True,
                        skip_group_check=True)
                src = tp.rearrange("p (b q) -> p b q", b=2)
                if qb == 0:
                    nc.vector.tensor_copy(otT_of[cp][:, :, qslot, :], src)
                else:
                    nc.scalar.activation(otT_of[cp][:, :, qslot, :], src,
                                         AF.Copy)
            else:
                nc.sync.dma_start(otT_of[cp][:, :, qslot, :],
                                  oc_of[cp][:, qslot, :], transpose=True)
            if qb == 1 and c % 2 == 1:
                oc_of.pop(cp, None)

        def emit_outproj(cp, eo, qh):
            # out_proj partial for chunk-pair cp, qslot-half qh (256 q):
            # zT[eo-block, q-half].  qh0 (the even chunk's qslots) runs a
            # full 12 slots earlier than qh1 so the PE stays fed while the
            # odd chunk's P@V/norm/otT chain drains.  Evac spread over
            # ACT/DVE/Pool; zT DMA'd per 2-eo piece to shorten the tail.
            otT = otT_of[cp]
            # the last chunk-pair's outproj runs after all scores are done,
            # so it can rotate through the 4 idle score banks instead of
            # fighting over the 2 misc banks
            zpool = scorep if cp == N_QC // 2 - 1 else miscp
            ztag = "sc" if cp == N_QC // 2 - 1 else "misc"
            zps = zpool.tile([128, 256], f32, tag=ztag, name="zps")
            for cc in range(2):
                nc.tensor.matmul(
                    zps[:], wo_sb[:, cc, eo * 128:(eo + 1) * 128],
                    otT[:, cc, 2 * qh:2 * qh + 2, :],
                    start=(cc == 0), stop=(cc == 1), skip_group_check=True)
            if eo == 0 and qh == 0:
                zsb[cp] = zsbp.tile([128, 8, 2 * NQ], fp16, tag="zsb",
                                    name="zs")
            dst = zsb[cp][:, eo, qh * 256:(qh + 1) * 256]
            sel = (eo + 2 * qh) % 4
            if sel in (0, 2):
                nc.scalar.activation(dst, zps[:], AF.Copy)
            elif sel == 1:
                nc.vector.tensor_copy(dst, zps[:])
            else:
                nc.gpsimd.tensor_copy(dst, zps[:])
            if qh == 1 and (eo % 2 == 1 or eo >= 6):
                lo = eo if eo >= 6 else eo - 1
                nc.sync.dma_start(
                    zT.rearrange("(eo p) s -> p eo s", p=128)[
                        :, lo:eo + 1, cp * 2 * NQ:(cp + 1) * 2 * NQ],
                    zsb[cp][:, lo:eo + 1, :])
            if eo == 7 and qh == 1:
                otT_of.pop(cp, None)

        # ---- software pipeline -----------------------------------------
        # lead-in: k half0 projection (both col-blocks, k DMAs land first),
        # then the first q block (PE is in-order, so k-dep work goes first)
        proj_pair(kts, wk_sb, bk_sb, kh, 0)
        proj_pair(kts, wk_sb, bk_sb, kh, 1)
        proj_pair(qts, wq_sb, bq_sb, qh, 0)

        extras = {}

        def add_extra(cs, kt, fn):
            extras.setdefault((cs, kt), []).append(fn)

        # chunk0: k half1 at kt4/6 (needed by scores kt8+),
        #         V st0-7 at kt8-15, V st8-15 at step1 kt0-7 (needed by
        #         the PV(c0) sweeps which start at step1 slot 8)
        add_extra(0, 4, lambda: proj_pair(kts, wk_sb, bk_sb, kh, 2))
        add_extra(0, 6, lambda: proj_pair(kts, wk_sb, bk_sb, kh, 3))
        for i in range(8):
            add_extra(0, 8 + i, lambda st=i: proj_v_st(st))
        for i in range(8):
            add_extra(1, i, lambda st=8 + i: proj_v_st(st))
        add_extra(1, 8, lambda: proj_pair(qts, wq_sb, bq_sb, qh, 1))
        add_extra(2, 5, lambda: proj_pair(qts, wq_sb, bq_sb, qh, 2))
        add_extra(3, 5, lambda: proj_pair(qts, wq_sb, bq_sb, qh, 3))

        # Schedule per step/slot.  ALL sweeps of chunk c run at step c+1
        # slots 8-15 (sweep s = kt-8, both halves); norms trail one slot
        # (s 0-6 at kt 9-15, s 7 at step c+2 kt 0); otT XBAR-transpose DMAs
        # at step c+2 slots 1/2; out_proj per chunk-pair at the odd chunk's
        # step+2, slots 6-13 (after the otT DMA latency), zT DMA'd per
        # 2-eo piece as rows complete.
        for step in range(N_QC + 2):
            for kt in range(N_KT):
                # trailing norm + otT + out_proj for chunk step-2
                c2 = step - 2
                if 0 <= c2 <= N_QC - 1:
                    if kt == 0:
                        emit_norm_s(c2, 7)
                        pts_of.pop(c2, None)
                    if kt == 1:
                        emit_otT_dma(c2, 1)
                    if c2 % 2 == 1:
                        if kt <= 3:
                            emit_outproj(c2 // 2, 2 * kt, 0)
                            emit_outproj(c2 // 2, 2 * kt + 1, 0)
                        if 6 <= kt <= 13:
                            emit_outproj(c2 // 2, kt - 6, 1)
                # sweeps + trailing norms for chunk step-1
                c1 = step - 1
                if 0 <= c1 <= N_QC - 1:
                    if kt >= 8:
                        emit_pv_half(c1, kt - 8, 0)
                        emit_pv_half(c1, kt - 8, 1)
                    if kt >= 9:
                        emit_norm_s(c1, kt - 9)
                    if kt == 14:
                        emit_otT_dma(c1, 0)
                for fn in extras.get((step, kt), ()):
                    fn()
                if step <= N_QC - 1:
                    emit_scores_exp_kt(step, kt)

    nc.compile()
    return nc


def _get_program():
    global _PROGRAM
    if _PROGRAM is None:
        _PROGRAM = _build_program()
    return _PROGRAM


def _make_in_maps(q, k, v, Wq, bq, Wk, bk, Wv, bv, Wo):
    f32 = np.float32
    xT = {}
    for b in range(B):
        xT[("q", b)] = np.ascontiguousarray(q[b].T, dtype=np.float16)
        xT[("k", b)] = np.ascontiguousarray(k[b].T, dtype=np.float16)
        xT[("v", b)] = np.ascontiguousarray(v[b].T, dtype=np.float16)
    wslices = {}
    for g in range(4):
        sl = slice(g * E, (g + 1) * E)
        wslices[("wq", g)] = np.ascontiguousarray(Wq[sl, :].T, dtype=np.float16)
        wslices[("wk", g)] = np.ascontiguousarray(Wk[sl, :].T, dtype=np.float16)
        wslices[("wv", g)] = np.ascontiguousarray(Wv[sl, :].T, dtype=np.float16)
        wslices[("wo", g)] = np.ascontiguousarray(Wo[:, sl].T, dtype=np.float16)
        wslices[("bq", g)] = np.ascontiguousarray(bq[sl].reshape(E, 1), dtype=f32)
        wslices[("bk", g)] = np.ascontiguousarray(bk[sl].reshape(E, 1), dtype=f32)
        wslices[("bvb", g)] = np.ascontiguousarray(
            np.tile(bv[sl].reshape(1, E), (128, 1)), dtype=f32)
    in_maps = []
    for c in range(N_CORES):
        b, g = c // 4, c % 4
        in_maps.append({
            "qT": xT[("q", b)], "kT": xT[("k", b)], "vT": xT[("v", b)],
            "wq": wslices[("wq", g)], "wk": wslices[("wk", g)],
            "wv": wslices[("wv", g)], "wo": wslices[("wo", g)],
            "bq": wslices[("bq", g)], "bk": wslices[("bk", g)],
            "bvb": wslices[("bvb", g)],
        })
    return in_maps


def _numpy_fallback(q, k, v, mask, Wq, bq, Wk, bk, Wv, bv, Wo, bo):
    # Only used if mask is not all-True (never the case for this problem).
    def proj(x, W, b_):
        y = x @ W.T + b_
        return y.reshape(B, S, NUM_HEADS, DK).transpose(0, 2, 1, 3)
    qh, kh, vh = proj(q, Wq, bq), proj(k, Wk, bk), proj(v, Wv, bv)
    sc = np.einsum("bhqd,bhkd->bhqk", qh, kh) / np.sqrt(DK)
    sc = np.where(mask, sc, np.float32(-1e9))
    sc = sc - sc.max(-1, keepdims=True)
    p = np.exp(sc)
    p /= p.sum(-1, keepdims=True)
    o = np.einsum("bhqk,bhkd->bhqd", p, vh)
    o = o.transpose(0, 2, 1, 3).reshape(B, S, D_MODEL)
    return (o @ Wo.T + bo).astype(np.float32)


def kernel(q, k, v, mask, Wq, bq, Wk, bk, Wv, bv, Wo, bo):
    q = np.asarray(q, dtype=np.float32)
    k = np.asarray(k, dtype=np.float32)
    v = np.asarray(v, dtype=np.float32)
    Wq, Wk, Wv, Wo = (np.asarray(w, dtype=np.float32) for w in (Wq, Wk, Wv, Wo))
    bq, bk, bv, bo = (np.asarray(x, dtype=np.float32) for x in (bq, bk, bv, bo))
    if not np.all(np.asarray(mask)):
        return _numpy_fallback(q, k, v, np.asarray(mask), Wq, bq, Wk, bk,
                               Wv, bv, Wo, bo)

    from concourse.bass_utils import run_bass_kernel_spmd
    nc = _get_program()
    in_maps = _make_in_maps(q, k, v, Wq, bq, Wk, bk, Wv, bv, Wo)
    res = run_bass_kernel_spmd(nc, in_maps, core_ids=list(range(N_CORES)),
                               **_RUN_KWARGS)
    global _LAST_RESULTS
    _LAST_RESULTS = res
    out = np.empty((B, S, D_MODEL), dtype=np.float32)
    for b in range(B):
        acc = res.results[4 * b]["zT"].astype(np.float32)
        for g in range(1, 4):
            acc = acc + res.results[4 * b + g]["zT"].astype(np.float32)
        out[b] = acc.T + bo
    return out
